# revision 1
# baseline (speedup 1.0000x reference)
"""Trainium2 Bass kernel for nn_Decoder (4-layer dense transformer decoder).

Sharding (8 NeuronCores):
  - Sequence-parallel residual stream: core c owns tokens [256c, 256c+256).
  - Attention is tensor-parallel over heads (2 heads/core); the normed
    activations are AllGathered (bf16, 4MB) once per layer; attention
    outputs return to token-local form via a small AllToAll (0.5MB).
  - FFN / out-proj / logits weights are replicated (bf16) and streamed from
    HBM, fully overlapped with compute; qkv weights are head-sharded.
  - Gamma, sqrt(D) and the attention 1/sqrt(dh) scale are folded into
    weights on the host; biases ride the ACT activation ops.

Layout: everything on-chip is transposed — [D(partitions), tokens(free)] —
so RMS-norm scaling, gelu bias and per-channel ops are native per-partition
ops, and matmul lhsT slices come straight from the weight matrices.
"""
import numpy as np
import ml_dtypes

import concourse.bass as bass
import concourse.mybir as mybir
import concourse.tile as tile
from concourse import bacc
from concourse.tile import TileContext
from concourse.masks import make_identity

BF16 = np.dtype(ml_dtypes.bfloat16)
AF = mybir.ActivationFunctionType
P = 128

# model dims
V, D, DEPTH, H, DH, FF = 32000, 1024, 4, 16, 64, 4096
B, N = 1, 2048
NC = 8  # cores


class Cfg:
    def __init__(self, n=N, depth=DEPTH, v=V, ff=FF):
        self.n = n            # total tokens
        self.depth = depth
        self.v = v
        self.ff = ff
        self.t = n // NC      # tokens per core
        self.tb = self.t // P           # token blocks per core
        self.db = D // P                # D blocks (8)
        self.fb = ff // P               # FF blocks (32)
        self.heads_per_core = H // NC   # 2
        self.hd = self.heads_per_core * DH  # 128 head-dims per core
        self.n_kb = n // P              # key blocks (16)
        self.qc_w = min(512, n)         # query chunk width
        self.n_qc = n // self.qc_w      # query chunks
        self.kb_per_qc = self.qc_w // P  # 4


FULL = Cfg()


def build_kernel(cfg=FULL):
    n, t, depth, v, ff = cfg.n, cfg.t, cfg.depth, cfg.v, cfg.ff
    db, fb, tb = cfg.db, cfg.fb, cfg.tb
    hd = cfg.hd
    f32, bf16, i32 = mybir.dt.float32, mybir.dt.bfloat16, mybir.dt.int32

    nc = bacc.Bacc(None, target_bir_lowering=False, debug=False, num_devices=NC)

    # ---------- DRAM I/O ----------
    emb_idx = nc.dram_tensor("emb_idx", [t, 1], i32, kind="ExternalInput")
    emb_tab = nc.dram_tensor("emb_tab", [v, D], f32, kind="ExternalInput")
    wqkv = nc.dram_tensor("wqkv", [depth, 3, D, hd], bf16, kind="ExternalInput")
    wout = nc.dram_tensor("wout", [depth, D, D], bf16, kind="ExternalInput")
    wff1 = nc.dram_tensor("wff1", [depth, D, ff], bf16, kind="ExternalInput")
    bff1 = nc.dram_tensor("bff1", [depth, P, fb], f32, kind="ExternalInput")
    wff2 = nc.dram_tensor("wff2", [depth, ff, D], bf16, kind="ExternalInput")
    bff2 = nc.dram_tensor("bff2", [depth, P, db], f32, kind="ExternalInput")
    wlog = nc.dram_tensor("wlog", [D, v], bf16, kind="ExternalInput")
    cosq = nc.dram_tensor("cosq", [P, n], bf16, kind="ExternalInput")
    sinq = nc.dram_tensor("sinq", [P, n], bf16, kind="ExternalInput")
    rotPT = nc.dram_tensor("rotPT", [P, P], bf16, kind="ExternalInput")
    # diagonal causal masks for the 512-wide q-chunk: j = kb offset in chunk
    trimask = nc.dram_tensor("trimask", [cfg.kb_per_qc, P, cfg.qc_w], bf16,
                             kind="ExternalInput")
    logits_out = nc.dram_tensor("logits_out", [t, v], f32, kind="ExternalOutput")

    # collective bounce buffers (reused across layers)
    ag_in = nc.dram_tensor("ag_in", [D, t], bf16)
    ag_out = nc.dram_tensor("ag_out", [NC, D, t], bf16, addr_space="Shared")
    a2a_in = nc.dram_tensor("a2a_in", [NC, hd, t], bf16)
    a2a_out = nc.dram_tensor("a2a_out", [NC, hd, t], bf16)

    with TileContext(nc) as tc:
        with tc.tile_pool(name="const", bufs=1) as cpool, \
             tc.tile_pool(name="resid", bufs=1) as rpool, \
             tc.tile_pool(name="work", bufs=1) as wpool, \
             tc.tile_pool(name="wts", bufs=2) as wtpool, \
             tc.tile_pool(name="small", bufs=3) as spool, \
             tc.tile_pool(name="attn", bufs=1) as apool, \
             tc.tile_pool(name="big", bufs=1) as bigpool, \
             tc.tile_pool(name="pexp", bufs=3) as epool, \
             tc.tile_pool(name="psum_acc", bufs=1, space="PSUM") as pacc, \
             tc.tile_pool(name="psum_s", bufs=4, space="PSUM") as ps:

            # ---------- constants ----------
            ident = cpool.tile([P, P], f32)
            make_identity(nc, ident[:])
            ones_bf = cpool.tile([P, 1], bf16)
            nc.vector.memset(ones_bf[:], 1.0)
            ones_row = cpool.tile([1, P], f32)
            nc.vector.memset(ones_row[:], 1.0)
            cos_t = cpool.tile([P, n], bf16)
            sin_t = cpool.tile([P, n], bf16)
            rot_t = cpool.tile([P, P], bf16)
            nc.sync.dma_start(cos_t[:], cosq[:, :])
            nc.sync.dma_start(sin_t[:], sinq[:, :])
            nc.sync.dma_start(rot_t[:], rotPT[:, :])
            mask_t = cpool.tile([P, cfg.kb_per_qc, cfg.qc_w], bf16)
            nc.sync.dma_start(
                mask_t[:], trimask[:, :, :].rearrange("j p q -> p j q"))

            # ---------- embedding gather -> hT [P, db, t] f32 ----------
            hT = rpool.tile([P, db, t], f32)
            for tbi in range(tb):
                idx_t = spool.tile([P, 1], i32, tag="idx")
                nc.sync.dma_start(idx_t[:], emb_idx[tbi * P:(tbi + 1) * P, :])
                g_t = wpool.tile([P, D], f32, tag="gather")
                nc.gpsimd.indirect_dma_start(
                    out=g_t[:], out_offset=None, in_=emb_tab[:, :],
                    in_offset=bass.IndirectOffsetOnAxis(ap=idx_t[:, :1], axis=0))
                for dbi in range(db):
                    ptr = ps.tile([P, P], f32, tag="ps")
                    nc.tensor.transpose(ptr[:], g_t[:, dbi * P:(dbi + 1) * P],
                                        ident[:])
                    nc.any.tensor_copy(
                        hT[:, dbi, tbi * P:(tbi + 1) * P], ptr[:])

            def rms_norm_cast(src_f32, dst_bf):
                """dst_bf[P, db, t] = src * rsqrt(sum_D(src^2)); sqrt(D)*gamma
                is folded into the consuming weights."""
                sq = wpool.tile([P, db, t], bf16, tag="normsq")
                for dbi in range(db):
                    nc.vector.tensor_tensor(
                        sq[:, dbi, :], src_f32[:, dbi, :], src_f32[:, dbi, :],
                        mybir.AluOpType.mult)
                psum_n = pacc.tile([1, t], f32, tag="acc_a")
                for dbi in range(db):
                    nc.tensor.matmul(psum_n[:], ones_bf[:], sq[:, dbi, :],
                                     start=(dbi == 0), stop=(dbi == db - 1))
                rt = spool.tile([1, t], f32, tag="norm_rt")
                nc.scalar.activation(rt[:], psum_n[:], AF.Sqrt)
                inv = spool.tile([1, t], f32, tag="norm_inv")
                nc.vector.reciprocal(inv[:], rt[:])
                psum_b = ps.tile([P, t], f32, tag="ps")
                nc.tensor.matmul(psum_b[:], ones_row[:], inv[:],
                                 start=True, stop=True)
                invb = spool.tile([P, t], f32, tag="norm_invb")
                nc.vector.tensor_copy(invb[:], psum_b[:])
                for dbi in range(db):
                    nc.vector.tensor_tensor(
                        dst_bf[:, dbi, :], src_f32[:, dbi, :], invb[:],
                        mybir.AluOpType.mult)

            # ================= layers =================
            for l in range(depth):
                # ----- norm1 + AllGather -----
                xn = wpool.tile([P, db, t], bf16, tag="xn")
                rms_norm_cast(hT, xn)
                nc.sync.dma_start(
                    ag_in[:, :].rearrange("(o p) t -> p o t", p=P), xn[:])
                nc.gpsimd.collective_compute(
                    "AllGather", mybir.AluOpType.bypass,
                    replica_groups=[list(range(NC))],
                    ins=[ag_in.ap().opt()], outs=[ag_out.ap().opt()])

                # xnT_global [P, db, n] bf16
                xg = bigpool.tile([P, db, n], bf16, tag="xg")
                for dbi in range(db):
                    nc.sync.dma_start(
                        xg[:, dbi, :].rearrange("d (c t) -> d c t", c=NC),
                        ag_out[:, dbi * P:(dbi + 1) * P, :]
                        .rearrange("c d t -> d c t"))

                # ----- qkv weights (head shard) -----
                wq = wtpool.tile([P, db, hd], bf16, tag="wq")
                wk = wtpool.tile([P, db, hd], bf16, tag="wk")
                wv = wtpool.tile([P, db, hd], bf16, tag="wv")
                for s, wt in ((0, wq), (1, wk), (2, wv)):
                    nc.sync.dma_start(
                        wt[:], wqkv[l, s, :, :].rearrange(
                            "(o p) c -> p o c", p=P))

                # ----- qT / kT with rope, V token-major -----
                qT = apool.tile([P, n], bf16, tag="qT")
                kT = apool.tile([P, n], bf16, tag="kT")
                for dst, wt, do_scale in ((qT, wq, True), (kT, wk, False)):
                    for tc_i in range(cfg.n_qc):
                        sl = slice(tc_i * cfg.qc_w, (tc_i + 1) * cfg.qc_w)
                        pq = ps.tile([P, cfg.qc_w], f32, tag="ps")
                        for dbi in range(db):
                            nc.tensor.matmul(pq[:], wt[:, dbi, :],
                                             xg[:, dbi, sl],
                                             start=(dbi == 0),
                                             stop=(dbi == db - 1))
                        raw = epool.tile([P, cfg.qc_w], bf16, tag="rope_raw")
                        nc.vector.tensor_copy(raw[:], pq[:])
                        prot = ps.tile([P, cfg.qc_w], f32, tag="ps")
                        nc.tensor.matmul(prot[:], rot_t[:], raw[:],
                                         start=True, stop=True)
                        t1 = epool.tile([P, cfg.qc_w], bf16, tag="rope_t1")
                        nc.vector.tensor_tensor(
                            t1[:], prot[:], sin_t[:, sl], mybir.AluOpType.mult)
                        t2 = epool.tile([P, cfg.qc_w], bf16, tag="rope_t2")
                        nc.vector.tensor_tensor(
                            t2[:], raw[:], cos_t[:, sl], mybir.AluOpType.mult)
                        nc.vector.tensor_tensor(
                            dst[:, sl], t1[:], t2[:], mybir.AluOpType.add)
                vtile = apool.tile([P, cfg.n_kb, hd], bf16, tag="V")
                for kbi in range(cfg.n_kb):
                    pv = ps.tile([P, hd], f32, tag="ps")
                    for dbi in range(db):
                        nc.tensor.matmul(
                            pv[:], xg[:, dbi, kbi * P:(kbi + 1) * P],
                            wv[:, dbi, :],
                            start=(dbi == 0), stop=(dbi == db - 1))
                    nc.vector.tensor_copy(vtile[:, kbi, :], pv[:])

                # ----- attention (2 heads), output attT [P, n] bf16 -----
                attT = apool.tile([P, n], bf16, tag="attT")
                for h in range(cfg.heads_per_core):
                    hsl = slice(h * DH, (h + 1) * DH)
                    for qc_i in range(cfg.n_qc):
                        qsl = slice(qc_i * cfg.qc_w, (qc_i + 1) * cfg.qc_w)
                        n_kb_q = (qc_i + 1) * cfg.kb_per_qc
                        pav = pacc.tile([DH, cfg.qc_w], f32, tag="acc_a")
                        psum = pacc.tile([1, cfg.qc_w], f32, tag="acc_b")
                        for kbi in range(n_kb_q):
                            pscr = ps.tile([P, cfg.qc_w], f32, tag="ps")
                            nc.tensor.matmul(
                                pscr[:], kT[hsl, kbi * P:(kbi + 1) * P],
                                qT[hsl, qsl], start=True, stop=True)
                            pe = epool.tile([P, cfg.qc_w], bf16, tag="att_exp")
                            nc.scalar.activation(pe[:], pscr[:], AF.Exp)
                            j = kbi - qc_i * cfg.kb_per_qc
                            if j >= 0:
                                nc.vector.tensor_tensor(
                                    pe[:], pe[:], mask_t[:, j, :],
                                    mybir.AluOpType.mult)
                            first, last = kbi == 0, kbi == n_kb_q - 1
                            nc.tensor.matmul(psum[:], ones_bf[:], pe[:],
                                             start=first, stop=last)
                            nc.tensor.matmul(pav[:], vtile[:, kbi, hsl], pe[:],
                                             start=first, stop=last)
                        inv = spool.tile([1, cfg.qc_w], f32, tag="att_inv")
                        nc.vector.reciprocal(inv[:], psum[:])
                        pb = ps.tile([DH, cfg.qc_w], f32, tag="ps")
                        nc.tensor.matmul(pb[:], ones_row[:, :DH], inv[:],
                                         start=True, stop=True)
                        invb = spool.tile([DH, cfg.qc_w], f32, tag="att_invb")
                        nc.vector.tensor_copy(invb[:], pb[:])
                        nc.vector.tensor_tensor(
                            attT[hsl, qsl], pav[:], invb[:],
                            mybir.AluOpType.mult)

                # ----- AllToAll back to token-local -----
                nc.sync.dma_start(
                    a2a_in[:, :, :].rearrange("c d t -> d c t"),
                    attT[:].rearrange("d (c t) -> d c t", c=NC))
                nc.gpsimd.collective_compute(
                    "AllToAll", mybir.AluOpType.bypass,
                    replica_groups=[list(range(NC))],
                    ins=[a2a_in.ap().opt()], outs=[a2a_out.ap().opt()])
                attC = wpool.tile([P, NC, t], bf16, tag="attC")
                nc.sync.dma_start(
                    attC[:], a2a_out[:, :, :].rearrange("c d t -> d c t"))

                # ----- out-proj + residual -----
                for dci in range(db):
                    woc = wtpool.tile([P, NC, P], bf16, tag="woc")
                    nc.sync.dma_start(
                        woc[:], wout[l, :, dci * P:(dci + 1) * P].rearrange(
                            "(hb p) q -> p hb q", p=P))
                    po = ps.tile([P, t], f32, tag="ps")
                    for hb in range(NC):
                        nc.tensor.matmul(po[:], woc[:, hb, :],
                                         attC[:, hb, :],
                                         start=(hb == 0), stop=(hb == NC - 1))
                    nc.vector.tensor_tensor(hT[:, dci, :], hT[:, dci, :],
                                            po[:], mybir.AluOpType.add)

                # ----- norm2 + FFN (token-local, no collective) -----
                xn2 = wpool.tile([P, db, t], bf16, tag="xn")
                rms_norm_cast(hT, xn2)
                b1 = spool.tile([P, fb], f32, tag="b1")
                nc.sync.dma_start(b1[:], bff1[l, :, :])
                b2 = spool.tile([P, db], f32, tag="b2")
                nc.sync.dma_start(b2[:], bff2[l, :, :])
                # fused ff1 -> gelu -> ff2: 4 pair-psum accumulators hold the
                # 8 D-chunk outputs; act chunk is transient.
                pgs = []
                for pi in range(4):
                    pg_i = pacc.tile([P, 2, t], f32, tag=f"acc_{'abcd'[pi]}",
                                     name=f"ffacc_{l}_{pi}")
                    pgs.append(pg_i)
                for fci in range(fb):
                    w1c = wtpool.tile([P, db, P], bf16, tag="w1c")
                    nc.sync.dma_start(
                        w1c[:], wff1[l, :, fci * P:(fci + 1) * P].rearrange(
                            "(o p) c -> p o c", p=P))
                    pf = ps.tile([P, t], f32, tag="ps")
                    for dbi in range(db):
                        nc.tensor.matmul(
                            pf[:], w1c[:, dbi, :], xn2[:, dbi, :],
                            start=(dbi == 0), stop=(dbi == db - 1))
                    act_c = epool.tile([P, t], bf16, tag="act_c")
                    nc.scalar.activation(act_c[:], pf[:], AF.Gelu,
                                         bias=b1[:, fci:fci + 1])
                    w2c = wtpool.tile([P, db, P], bf16, tag="w2c")
                    nc.sync.dma_start(
                        w2c[:], wff2[l, fci * P:(fci + 1) * P, :].rearrange(
                            "p (dc q) -> p dc q", q=P))
                    for dci in range(db):
                        nc.tensor.matmul(
                            pgs[dci // 2][:, dci % 2, :], w2c[:, dci, :],
                            act_c[:],
                            start=(fci == 0 and dci % 2 == 0),
                            stop=(fci == fb - 1 and dci % 2 == 1))
                for dci in range(db):
                    tmp = spool.tile([P, t], f32, tag="ff2_tmp")
                    nc.scalar.activation(tmp[:], pgs[dci // 2][:, dci % 2, :],
                                         AF.Identity, bias=b2[:, dci:dci + 1])
                    nc.vector.tensor_tensor(hT[:, dci, :], hT[:, dci, :],
                                            tmp[:], mybir.AluOpType.add)

            # ================= final norm + logits =================
            xnf = wpool.tile([P, db, t], bf16, tag="xn")
            rms_norm_cast(hT, xnf)
            vchunks = []
            off = 0
            while off < v:
                w = min(512, v - off)
                vchunks.append((off, w))
                off += w
            for (off, w) in vchunks:
                wl = wtpool.tile([P, db, 512], bf16, tag="wl")
                nc.sync.dma_start(
                    wl[:, :, :w],
                    wlog[:, off:off + w].rearrange("(o p) c -> p o c", p=P))
                for tbi in range(tb):
                    pl = pacc.tile([P, 512], f32, tag=f"acc_{'ab'[tbi % 2]}",
                                   name=f"pl_{off}_{tbi}")
                    for dbi in range(db):
                        nc.tensor.matmul(
                            pl[:, :w], xnf[:, dbi, tbi * P:(tbi + 1) * P],
                            wl[:, dbi, :w],
                            start=(dbi == 0), stop=(dbi == db - 1))
                    ot = spool.tile([P, 512], f32, tag="log_out")
                    nc.vector.tensor_copy(ot[:, :w], pl[:, :w])
                    nc.sync.dma_start(
                        logits_out[tbi * P:(tbi + 1) * P, off:off + w],
                        ot[:, :w])
    nc.finalize()
    return nc


# ======================= host side =======================

def prep_inputs(inputs, cfg=FULL):
    """Full model inputs -> list of 8 per-core input dicts (numpy)."""
    n, t, depth, v, ff = cfg.n, cfg.t, cfg.depth, cfg.v, cfg.ff
    x = np.asarray(inputs["x"]).reshape(-1)[:n].astype(np.int32)
    emb = np.asarray(inputs["token_emb"], dtype=np.float32)[:v]
    attn_g = np.asarray(inputs["attn_gamma"], dtype=np.float32)
    w_qkv = np.asarray(inputs["w_qkv"], dtype=np.float32)
    w_out = np.asarray(inputs["w_attn_out"], dtype=np.float32)
    ff_g = np.asarray(inputs["ff_gamma"], dtype=np.float32)
    w_ff1 = np.asarray(inputs["w_ff1"], dtype=np.float32)[:, :, :ff]
    b_ff1 = np.asarray(inputs["b_ff1"], dtype=np.float32)[:, :ff]
    w_ff2 = np.asarray(inputs["w_ff2"], dtype=np.float32)[:, :ff, :]
    b_ff2 = np.asarray(inputs["b_ff2"], dtype=np.float32)
    fin_g = np.asarray(inputs["final_gamma"], dtype=np.float32)
    w_log = np.asarray(inputs["w_logits"], dtype=np.float32)[:, :v]

    sD = float(np.sqrt(D))
    # rope tables (constants of shape only)
    inv_freq = 1.0 / (10000.0 ** (np.arange(0, DH, 2, dtype=np.float32) / DH))
    freqs = np.arange(n, dtype=np.float32)[:, None] * inv_freq[None, :]
    freqs = np.repeat(freqs, 2, axis=-1)          # [n, DH]
    cos = np.cos(freqs).T                          # [DH, n]
    sin = np.sin(freqs).T
    cos2 = np.tile(cos, (cfg.heads_per_core, 1)).astype(BF16)   # [128, n]
    sin2 = np.tile(sin, (cfg.heads_per_core, 1)).astype(BF16)
    # rot matrix PT st rot(x) = PT.T @ x, block-diag per head (DH x DH)
    Pm = np.zeros((DH, DH), np.float32)
    for i in range(0, DH, 2):
        Pm[i, i + 1] = -1.0
        Pm[i + 1, i] = 1.0
    PT1 = Pm.T
    PT = np.zeros((P, P), np.float32)
    for h in range(cfg.heads_per_core):
        PT[h * DH:(h + 1) * DH, h * DH:(h + 1) * DH] = PT1
    PT = PT.astype(BF16)
    # causal masks for diagonal kb of each q-chunk
    tri = np.zeros((cfg.kb_per_qc, P, cfg.qc_w), np.float32)
    qpos = np.arange(cfg.qc_w)
    for j in range(cfg.kb_per_qc):
        kpos = j * P + np.arange(P)
        tri[j] = (kpos[:, None] <= qpos[None, :]).astype(np.float32)
    tri = tri.astype(BF16)

    scale_q = DH ** -0.5
    in_maps = []
    for c in range(NC):
        heads = slice(c * cfg.heads_per_core * DH, (c + 1) * cfg.heads_per_core * DH)
        wq_l, wo_l, w1_l, w2_l = [], [], [], []
        b1_l, b2_l = [], []
        for l in range(cfg.depth):
            g = attn_g[l] * sD
            wqkv3 = w_qkv[l].reshape(D, 3, H * DH)
            wq = (g[:, None] * wqkv3[:, 0, heads]) * scale_q
            wk = g[:, None] * wqkv3[:, 1, heads]
            wv = g[:, None] * wqkv3[:, 2, heads]
            wq_l.append(np.stack([wq, wk, wv], 0))
            wo_l.append(w_out[l])
            gf = ff_g[l] * sD
            w1_l.append(gf[:, None] * w_ff1[l])
            w2_l.append(w_ff2[l])
            b1_l.append(b_ff1[l].reshape(cfg.fb, P).T)
            b2_l.append(b_ff2[l].reshape(cfg.db, P).T)
        wlogg = (fin_g * sD)[:, None] * w_log
        in_maps.append({
            "emb_idx": x[c * t:(c + 1) * t].reshape(t, 1),
            "emb_tab": emb,
            "wqkv": np.stack(wq_l, 0).astype(BF16),
            "wout": np.stack(wo_l, 0).astype(BF16),
            "wff1": np.stack(w1_l, 0).astype(BF16),
            "bff1": np.stack(b1_l, 0).astype(np.float32),
            "wff2": np.stack(w2_l, 0).astype(BF16),
            "bff2": np.stack(b2_l, 0).astype(np.float32),
            "wlog": wlogg.astype(BF16),
            "cosq": cos2, "sinq": sin2, "rotPT": PT, "trimask": tri,
        })
    return in_maps


_CACHED = {}


def kernel(**inputs):
    import jax
    from jax.sharding import Mesh, PartitionSpec
    from jax.experimental.shard_map import shard_map
    from concourse.bass2jax import (_bass_exec_p, install_neuronx_cc_hook,
                                    partition_id_tensor)
    cfg = FULL
    in_maps = prep_inputs(inputs, cfg)
    if "nc" not in _CACHED:
        _CACHED["nc"] = build_kernel(cfg)
    nc = _CACHED["nc"]
    install_neuronx_cc_hook()
    partition_name = nc.partition_id_tensor.name if nc.partition_id_tensor else None
    in_names, out_names, out_avals = [], [], []
    for alloc in nc.m.functions[0].allocations:
        if not isinstance(alloc, mybir.MemoryLocationSet):
            continue
        name = alloc.memorylocations[0].name
        if alloc.kind == "ExternalInput":
            if name != partition_name:
                in_names.append(name)
        elif alloc.kind == "ExternalOutput":
            out_names.append(name)
            out_avals.append(jax.core.ShapedArray(
                tuple(alloc.tensor_shape), mybir.dt.np(alloc.dtype)))
    all_in = list(in_names) + list(out_names)
    if partition_name is not None:
        all_in.append(partition_name)
    n_params = len(in_names)

    def _body(*args):
        operands = list(args)
        if partition_name is not None:
            operands.append(partition_id_tensor())
        return tuple(_bass_exec_p.bind(
            *operands, out_avals=tuple(out_avals), in_names=tuple(all_in),
            out_names=tuple(out_names), lowering_input_output_aliases=(),
            sim_require_finite=True, sim_require_nnan=True, nc=nc))

    devices = jax.devices()[:NC]
    mesh = Mesh(np.asarray(devices), ("core",))
    n_outs = len(out_names)
    sharded = jax.jit(
        shard_map(_body, mesh=mesh,
                  in_specs=(PartitionSpec("core"),) * (n_params + n_outs),
                  out_specs=(PartitionSpec("core"),) * n_outs,
                  check_rep=False),
        donate_argnums=tuple(range(n_params, n_params + n_outs)),
        keep_unused=True)
    concat_in = [np.concatenate([np.asarray(in_maps[c][nm]) for c in range(NC)], 0)
                 for nm in in_names]
    zeros = [np.zeros((NC * a.shape[0], *a.shape[1:]), a.dtype) for a in out_avals]
    out = sharded(*concat_in, *zeros)
    logits = np.asarray(out[out_names.index("logits_out")])
    return logits.reshape(B, cfg.n, cfg.v).astype(np.float32)



# revision 9
# speedup vs baseline: 1.1902x; 1.1902x over previous
"""Trainium2 Bass kernel for nn_Decoder (4-layer dense transformer decoder).

Sharding (8 NeuronCores), v2:
  - Sequence-parallel residual stream: core c owns tokens [256c, 256c+256).
  - qkv is computed token-locally for ALL heads (full qkv weights are
    replicated, streamed bf16), with rope applied locally; a single
    AllToAll (1.5MiB out) redistributes q/k/v to head-parallel form
    (2 heads/core, all 2048 tokens). This replaces the v1 per-layer 4MiB
    AllGather (2.2x cheaper collective).
  - Attention is head-parallel; the softmax denominator rides the A@V
    matmul as a ones-column appended to V (no separate ones-matmul chain).
  - Attention outputs return to token-local form via a small AllToAll
    (0.5MiB); out-proj/FFN/logits run token-locally with replicated
    weights streamed from HBM in large contiguous chunks (>=1KB elements
    to dodge the sub-512B DMA read-modify-write penalty).
  - Gamma, sqrt(D) and the attention 1/sqrt(dh) scale are folded into
    weights on the host; biases ride the ACT activation ops.

Layout: everything on-chip is transposed — [D(partitions), tokens(free)].
"""
import numpy as np
import ml_dtypes

import concourse.bass as bass
import concourse.mybir as mybir
import concourse.tile as tile
from concourse import bacc
from concourse.tile import TileContext
from concourse.masks import make_identity

BF16 = np.dtype(ml_dtypes.bfloat16)
AF = mybir.ActivationFunctionType
P = 128

# model dims
V, D, DEPTH, H, DH, FF = 32000, 1024, 4, 16, 64, 4096
B, N = 1, 2048
NC = 8  # cores


class Cfg:
    def __init__(self, n=N, depth=DEPTH, v=V, ff=FF):
        self.n = n            # total tokens
        self.depth = depth
        self.v = v
        self.ff = ff
        self.t = n // NC      # tokens per core
        self.tb = self.t // P           # token blocks per core
        self.db = D // P                # D blocks (8)
        self.fb = ff // P               # FF blocks (32)
        self.heads_per_core = H // NC   # 2
        self.hd = self.heads_per_core * DH  # 128 head-dims per core
        self.n_kb = n // P              # key blocks (16)
        self.qc_w = min(512, n)         # query chunk width
        self.n_qc = n // self.qc_w      # query chunks
        self.kb_per_qc = self.qc_w // P  # 4


FULL = Cfg()


def build_kernel(cfg=FULL):
    n, t, depth, v, ff = cfg.n, cfg.t, cfg.depth, cfg.v, cfg.ff
    db, fb, tb = cfg.db, cfg.fb, cfg.tb
    hd = cfg.hd
    nqb = 3 * NC          # qkv partition blocks (24): dest-major [q,k,v]
    f32, bf16, i32 = mybir.dt.float32, mybir.dt.bfloat16, mybir.dt.int32

    nc = bacc.Bacc(None, target_bir_lowering=False, debug=False, num_devices=NC)

    # ---------- DRAM I/O ----------
    emb_idx = nc.dram_tensor("emb_idx", [t, 1], i32, kind="ExternalInput")
    emb_tab = nc.dram_tensor("emb_tab", [v, D], f32, kind="ExternalInput")
    # full qkv weights, columns regrouped dest-major: block 3c+j (j=q,k,v)
    wqkv = nc.dram_tensor("wqkv", [depth, D, nqb * P], bf16, kind="ExternalInput")
    wout = nc.dram_tensor("wout", [depth, D, D], bf16, kind="ExternalInput")
    wff1 = nc.dram_tensor("wff1", [depth, D, ff], bf16, kind="ExternalInput")
    bff1 = nc.dram_tensor("bff1", [depth, P, fb], f32, kind="ExternalInput")
    wff2 = nc.dram_tensor("wff2", [depth, ff, D], bf16, kind="ExternalInput")
    bff2 = nc.dram_tensor("bff2", [depth, P, db], f32, kind="ExternalInput")
    wlog = nc.dram_tensor("wlog", [D, v], bf16, kind="ExternalInput")
    cosq = nc.dram_tensor("cosq", [P, t], bf16, kind="ExternalInput")
    sinq = nc.dram_tensor("sinq", [P, t], bf16, kind="ExternalInput")
    rotPT = nc.dram_tensor("rotPT", [P, P], bf16, kind="ExternalInput")
    # diagonal causal masks for the 512-wide q-chunk: j = kb offset in chunk
    trimask = nc.dram_tensor("trimask", [cfg.kb_per_qc, P, cfg.qc_w], bf16,
                             kind="ExternalInput")
    logits_out = nc.dram_tensor("logits_out", [t, v], f32, kind="ExternalOutput")

    # collective bounce buffers (reused across layers)
    a2a_in = nc.dram_tensor("a2a_in", [NC, 3 * P, t], bf16)
    a2a_out = nc.dram_tensor("a2a_out", [NC, 3 * P, t], bf16)
    a2o_in = nc.dram_tensor("a2o_in", [NC, hd, t], bf16)
    a2o_out = nc.dram_tensor("a2o_out", [NC, hd, t], bf16)

    with TileContext(nc) as tc:
        with tc.tile_pool(name="const", bufs=1) as cpool, \
             tc.tile_pool(name="resid", bufs=1) as rpool, \
             tc.tile_pool(name="work", bufs=1) as wpool, \
             tc.tile_pool(name="wts", bufs=2) as wtpool, \
             tc.tile_pool(name="qkvw", bufs=2) as qwpool, \
             tc.tile_pool(name="small", bufs=3) as spool, \
             tc.tile_pool(name="attn", bufs=1) as apool, \
             tc.tile_pool(name="pexp", bufs=3) as epool, \
             tc.tile_pool(name="psum_acc", bufs=1, space="PSUM") as pacc, \
             tc.tile_pool(name="psum_s", bufs=4, space="PSUM") as ps:

            # ---------- constants ----------
            ident = cpool.tile([P, P], f32)
            make_identity(nc, ident[:])
            ident_bf = cpool.tile([P, P], bf16)
            nc.vector.tensor_copy(ident_bf[:], ident[:])
            ones_bf = cpool.tile([P, 1], bf16)
            nc.vector.memset(ones_bf[:], 1.0)
            ones_row = cpool.tile([1, P], f32)
            nc.vector.memset(ones_row[:], 1.0)
            cos_t = cpool.tile([P, t], bf16)
            sin_t = cpool.tile([P, t], bf16)
            rot_t = cpool.tile([P, P], bf16)
            nc.sync.dma_start(cos_t[:], cosq[:, :])
            nc.sync.dma_start(sin_t[:], sinq[:, :])
            nc.sync.dma_start(rot_t[:], rotPT[:, :])
            mask_t = cpool.tile([P, cfg.kb_per_qc, cfg.qc_w], bf16)
            nc.sync.dma_start(
                mask_t[:], trimask[:, :, :].rearrange("j p q -> p j q"))
            # V tiles (per head): col 64 stays all-ones for the softmax
            # denominator; cols 0:64 rewritten each layer.
            vh = []
            for h in range(cfg.heads_per_core):
                vt = cpool.tile([P, cfg.n_kb, DH + 1], bf16, name=f"vh{h}")
                nc.vector.memset(vt[:], 1.0)
                vh.append(vt)

            # ---------- embedding gather -> hT [P, db, t] f32 ----------
            hT = rpool.tile([P, db, t], f32)
            for tbi in range(tb):
                idx_t = spool.tile([P, 1], i32, tag="idx")
                nc.sync.dma_start(idx_t[:], emb_idx[tbi * P:(tbi + 1) * P, :])
                g_t = wpool.tile([P, D], f32, tag="gather")
                nc.gpsimd.indirect_dma_start(
                    out=g_t[:], out_offset=None, in_=emb_tab[:, :],
                    in_offset=bass.IndirectOffsetOnAxis(ap=idx_t[:, :1], axis=0))
                for dbi in range(db):
                    ptr = ps.tile([P, P], f32, tag="ps")
                    nc.tensor.transpose(ptr[:], g_t[:, dbi * P:(dbi + 1) * P],
                                        ident[:])
                    nc.any.tensor_copy(
                        hT[:, dbi, tbi * P:(tbi + 1) * P], ptr[:])

            def rms_norm_cast(src_f32, dst_bf):
                """dst_bf[P, db, t] = src * rsqrt(sum_D(src^2)); sqrt(D)*gamma
                is folded into the consuming weights."""
                sq = wpool.tile([P, db, t], bf16, tag="normsq")
                for dbi in range(db):
                    nc.vector.tensor_tensor(
                        sq[:, dbi, :], src_f32[:, dbi, :], src_f32[:, dbi, :],
                        mybir.AluOpType.mult)
                psum_n = pacc.tile([1, t], f32, tag="acc_a")
                for dbi in range(db):
                    nc.tensor.matmul(psum_n[:], ones_bf[:], sq[:, dbi, :],
                                     start=(dbi == 0), stop=(dbi == db - 1))
                rt = spool.tile([1, t], f32, tag="norm_rt")
                nc.scalar.activation(rt[:], psum_n[:], AF.Sqrt)
                inv = spool.tile([1, t], f32, tag="norm_inv")
                nc.vector.reciprocal(inv[:], rt[:])
                psum_b = ps.tile([P, t], f32, tag="ps")
                nc.tensor.matmul(psum_b[:], ones_row[:], inv[:],
                                 start=True, stop=True)
                invb = spool.tile([P, t], f32, tag="norm_invb")
                nc.vector.tensor_copy(invb[:], psum_b[:])
                for dbi in range(db):
                    nc.vector.tensor_tensor(
                        dst_bf[:, dbi, :], src_f32[:, dbi, :], invb[:],
                        mybir.AluOpType.mult)

            # ================= layers =================
            for l in range(depth):
                # ----- norm1 -----
                xn = wpool.tile([P, db, t], bf16, tag="xn")
                rms_norm_cast(hT, xn)

                # ----- token-local qkv (all heads) + rope, then AllToAll -----
                qkv_sb = wpool.tile([P, nqb, t], bf16, tag="qkv_sb")
                QW = nqb * P // 4     # qkv weight cols per load group (768)
                for quarter in range(4):
                    wq = qwpool.tile([P, db, QW], bf16, tag="wq")
                    nc.sync.dma_start(
                        wq[:], wqkv[l, :, quarter * QW:
                                     (quarter + 1) * QW].rearrange(
                            "(o p) c -> p o c", p=P))
                    for blk in range(nqb // 4):
                        b = quarter * (nqb // 4) + blk
                        comp = b % 3          # 0=q, 1=k, 2=v
                        pq = ps.tile([P, t], f32, tag="ps")
                        for dbi in range(db):
                            nc.tensor.matmul(
                                pq[:], wq[:, dbi, blk * P:(blk + 1) * P],
                                xn[:, dbi, :],
                                start=(dbi == 0), stop=(dbi == db - 1))
                        if comp == 2:
                            nc.vector.tensor_copy(qkv_sb[:, b, :], pq[:])
                        else:
                            raw = epool.tile([P, t], bf16, tag="rope_raw")
                            nc.vector.tensor_copy(raw[:], pq[:])
                            prot = ps.tile([P, t], f32, tag="ps")
                            nc.tensor.matmul(prot[:], rot_t[:], raw[:],
                                             start=True, stop=True)
                            t1 = epool.tile([P, t], bf16, tag="rope_t1")
                            nc.vector.tensor_tensor(
                                t1[:], prot[:], sin_t[:],
                                mybir.AluOpType.mult)
                            t2 = epool.tile([P, t], bf16, tag="rope_t2")
                            nc.vector.tensor_tensor(
                                t2[:], raw[:], cos_t[:],
                                mybir.AluOpType.mult)
                            nc.vector.tensor_tensor(
                                qkv_sb[:, b, :], t1[:], t2[:],
                                mybir.AluOpType.add)
                nc.sync.dma_start(
                    a2a_in[:, :, :].rearrange("c (j p) t -> p (c j) t", p=P),
                    qkv_sb[:])
                nc.gpsimd.collective_compute(
                    "AllToAll", mybir.AluOpType.bypass,
                    replica_groups=[list(range(NC))],
                    ins=[a2a_in.ap().opt()], outs=[a2a_out.ap().opt()])

                # ----- unpack: qT/kT [P, n]; V -> per-head [tok, dh|ones] -----
                qT = apool.tile([P, n], bf16, tag="qT")
                kT = apool.tile([P, n], bf16, tag="kT")
                vT = apool.tile([P, n], bf16, tag="vT")
                for dst, j in ((qT, 0), (kT, 1), (vT, 2)):
                    nc.sync.dma_start(
                        dst[:].rearrange("d (c t) -> d c t", c=NC),
                        a2a_out[:, j * P:(j + 1) * P, :]
                        .rearrange("c d t -> d c t"))
                for kbi in range(cfg.n_kb):
                    ptv = ps.tile([P, P], f32, tag="ps")
                    nc.tensor.matmul(
                        ptv[:], vT[:, kbi * P:(kbi + 1) * P], ident_bf[:],
                        start=True, stop=True)
                    for h in range(cfg.heads_per_core):
                        nc.vector.tensor_copy(
                            vh[h][:, kbi, :DH], ptv[:, h * DH:(h + 1) * DH])

                # ----- attention (2 heads), output attT [P, n] bf16 -----
                attT = apool.tile([P, n], bf16, tag="attT")
                for h in range(cfg.heads_per_core):
                    hsl = slice(h * DH, (h + 1) * DH)
                    for qc_i in range(cfg.n_qc):
                        qsl = slice(qc_i * cfg.qc_w, (qc_i + 1) * cfg.qc_w)
                        n_kb_q = (qc_i + 1) * cfg.kb_per_qc
                        pav = pacc.tile([DH + 1, cfg.qc_w], f32, tag="acc_a")
                        for kbi in range(n_kb_q):
                            pscr = ps.tile([P, cfg.qc_w], f32, tag="ps")
                            nc.tensor.matmul(
                                pscr[:], kT[hsl, kbi * P:(kbi + 1) * P],
                                qT[hsl, qsl], start=True, stop=True)
                            pe = epool.tile([P, cfg.qc_w], bf16, tag="att_exp")
                            nc.scalar.activation(pe[:], pscr[:], AF.Exp)
                            j = kbi - qc_i * cfg.kb_per_qc
                            if j >= 0:
                                nc.vector.tensor_tensor(
                                    pe[:], pe[:], mask_t[:, j, :],
                                    mybir.AluOpType.mult)
                            first, last = kbi == 0, kbi == n_kb_q - 1
                            nc.tensor.matmul(pav[:], vh[h][:, kbi, :], pe[:],
                                             start=first, stop=last)
                        inv = spool.tile([1, cfg.qc_w], f32, tag="att_inv")
                        nc.vector.reciprocal(inv[:], pav[DH:DH + 1, :])
                        pb = ps.tile([DH, cfg.qc_w], f32, tag="ps")
                        nc.tensor.matmul(pb[:], ones_row[:, :DH], inv[:],
                                         start=True, stop=True)
                        invb = spool.tile([DH, cfg.qc_w], f32, tag="att_invb")
                        nc.vector.tensor_copy(invb[:], pb[:])
                        nc.vector.tensor_tensor(
                            attT[hsl, qsl], pav[:DH, :], invb[:],
                            mybir.AluOpType.mult)

                # ----- AllToAll back to token-local -----
                nc.sync.dma_start(
                    a2o_in[:, :, :].rearrange("c d t -> d c t"),
                    attT[:].rearrange("d (c t) -> d c t", c=NC))
                nc.gpsimd.collective_compute(
                    "AllToAll", mybir.AluOpType.bypass,
                    replica_groups=[list(range(NC))],
                    ins=[a2o_in.ap().opt()], outs=[a2o_out.ap().opt()])
                attC = wpool.tile([P, NC, t], bf16, tag="attC")
                nc.sync.dma_start(
                    attC[:], a2o_out[:, :, :].rearrange("c d t -> d c t"))

                # ----- out-proj + residual -----
                # contraction split in two halves of 4 hb blocks so the
                # weight tile double-buffers at half size; 8 outputs live in
                # 4 pair-packed psum accumulators across both halves
                pgo = [pacc.tile([P, 2, t], f32, tag=f"acc_{'abcd'[pi]}",
                                 name=f"oacc_{l}_{pi}") for pi in range(4)]
                for half in range(2):
                    wo = wtpool.tile([P, NC // 2, D], bf16, tag="wo")
                    nc.sync.dma_start(
                        wo[:], wout[l, half * (D // 2):(half + 1) * (D // 2),
                                    :].rearrange("(hb p) q -> p hb q", p=P))
                    for dci in range(db):
                        for hb in range(NC // 2):
                            nc.tensor.matmul(
                                pgo[dci // 2][:, dci % 2, :],
                                wo[:, hb, dci * P:(dci + 1) * P],
                                attC[:, half * 4 + hb, :],
                                start=(half == 0 and hb == 0),
                                stop=(half == 1 and hb == NC // 2 - 1))
                for dci in range(db):
                    nc.vector.tensor_tensor(hT[:, dci, :], hT[:, dci, :],
                                            pgo[dci // 2][:, dci % 2, :],
                                            mybir.AluOpType.add)

                # ----- norm2 + FFN (token-local, no collective) -----
                xn2 = wpool.tile([P, db, t], bf16, tag="xn")
                rms_norm_cast(hT, xn2)
                b1 = spool.tile([P, fb], f32, tag="b1")
                nc.sync.dma_start(b1[:], bff1[l, :, :])
                b2 = spool.tile([P, db], f32, tag="b2")
                nc.sync.dma_start(b2[:], bff2[l, :, :])
                # fused ff1 -> gelu -> ff2: 4 pair-psum accumulators hold the
                # 8 D-chunk outputs; act chunk is transient. Weights stream in
                # 512-ff-col groups (contiguous >=1KB DMA elements).
                pgs = []
                for pi in range(4):
                    pg_i = pacc.tile([P, 2, t], f32, tag=f"acc_{'abcd'[pi]}",
                                     name=f"ffacc_{l}_{pi}")
                    pgs.append(pg_i)
                GW = 512            # ff cols per weight-load group
                ng = ff // GW
                fpg = GW // P       # fci per group (4)
                for gi in range(ng):
                    w1g = wtpool.tile([P, db, GW], bf16, tag="w1g")
                    nc.sync.dma_start(
                        w1g[:], wff1[l, :, gi * GW:(gi + 1) * GW].rearrange(
                            "(o p) c -> p o c", p=P))
                    w2g = wtpool.tile([P, fpg, db, P], bf16, tag="w2g")
                    nc.sync.dma_start(
                        w2g[:], wff2[l, gi * GW:(gi + 1) * GW, :].rearrange(
                            "(f p) (dc q) -> p f dc q", p=P, q=P))
                    for fi in range(fpg):
                        fci = gi * fpg + fi
                        pf = ps.tile([P, t], f32, tag="ps")
                        for dbi in range(db):
                            nc.tensor.matmul(
                                pf[:], w1g[:, dbi, fi * P:(fi + 1) * P],
                                xn2[:, dbi, :],
                                start=(dbi == 0), stop=(dbi == db - 1))
                        act_c = epool.tile([P, t], bf16, tag="act_c")
                        nc.scalar.activation(act_c[:], pf[:], AF.Gelu,
                                             bias=b1[:, fci:fci + 1])
                        for dci in range(db):
                            nc.tensor.matmul(
                                pgs[dci // 2][:, dci % 2, :],
                                w2g[:, fi, dci, :], act_c[:],
                                start=(fci == 0 and dci % 2 == 0),
                                stop=(fci == fb - 1 and dci % 2 == 1))
                for dci in range(db):
                    tmp = spool.tile([P, t], f32, tag="ff2_tmp")
                    nc.scalar.activation(tmp[:], pgs[dci // 2][:, dci % 2, :],
                                         AF.Identity, bias=b2[:, dci:dci + 1])
                    nc.vector.tensor_tensor(hT[:, dci, :], hT[:, dci, :],
                                            tmp[:], mybir.AluOpType.add)

            # ================= final norm + logits =================
            xnf = wpool.tile([P, db, t], bf16, tag="xn")
            rms_norm_cast(hT, xnf)
            vchunks = []
            off = 0
            while off < v:
                w = min(512, v - off)
                vchunks.append((off, w))
                off += w
            for (off, w) in vchunks:
                wl = wtpool.tile([P, db, 512], bf16, tag="wl")
                nc.sync.dma_start(
                    wl[:, :, :w],
                    wlog[:, off:off + w].rearrange("(o p) c -> p o c", p=P))
                for tbi in range(tb):
                    pl = pacc.tile([P, 512], f32, tag=f"acc_{'ab'[tbi % 2]}",
                                   name=f"pl_{off}_{tbi}")
                    for dbi in range(db):
                        nc.tensor.matmul(
                            pl[:, :w], xnf[:, dbi, tbi * P:(tbi + 1) * P],
                            wl[:, dbi, :w],
                            start=(dbi == 0), stop=(dbi == db - 1))
                    ot = spool.tile([P, 512], f32, tag="log_out")
                    nc.vector.tensor_copy(ot[:, :w], pl[:, :w])
                    nc.sync.dma_start(
                        logits_out[tbi * P:(tbi + 1) * P, off:off + w],
                        ot[:, :w])
    nc.finalize()
    return nc


# ======================= host side =======================

def prep_inputs(inputs, cfg=FULL):
    """Full model inputs -> list of 8 per-core input dicts (numpy)."""
    n, t, depth, v, ff = cfg.n, cfg.t, cfg.depth, cfg.v, cfg.ff
    x = np.asarray(inputs["x"]).reshape(-1)[:n].astype(np.int32)
    emb = np.asarray(inputs["token_emb"], dtype=np.float32)[:v]
    attn_g = np.asarray(inputs["attn_gamma"], dtype=np.float32)
    w_qkv = np.asarray(inputs["w_qkv"], dtype=np.float32)
    w_out = np.asarray(inputs["w_attn_out"], dtype=np.float32)
    ff_g = np.asarray(inputs["ff_gamma"], dtype=np.float32)
    w_ff1 = np.asarray(inputs["w_ff1"], dtype=np.float32)[:, :, :ff]
    b_ff1 = np.asarray(inputs["b_ff1"], dtype=np.float32)[:, :ff]
    w_ff2 = np.asarray(inputs["w_ff2"], dtype=np.float32)[:, :ff, :]
    b_ff2 = np.asarray(inputs["b_ff2"], dtype=np.float32)
    fin_g = np.asarray(inputs["final_gamma"], dtype=np.float32)
    w_log = np.asarray(inputs["w_logits"], dtype=np.float32)[:, :v]

    sD = float(np.sqrt(D))
    # rope tables
    inv_freq = 1.0 / (10000.0 ** (np.arange(0, DH, 2, dtype=np.float32) / DH))
    freqs = np.arange(n, dtype=np.float32)[:, None] * inv_freq[None, :]
    freqs = np.repeat(freqs, 2, axis=-1)          # [n, DH]
    cos = np.cos(freqs).T                          # [DH, n]
    sin = np.sin(freqs).T
    cos2 = np.tile(cos, (2, 1)).astype(BF16)       # [128, n]
    sin2 = np.tile(sin, (2, 1)).astype(BF16)
    # rot matrix PT st rot(x) = PT.T @ x, block-diag per head (DH x DH)
    Pm = np.zeros((DH, DH), np.float32)
    for i in range(0, DH, 2):
        Pm[i, i + 1] = -1.0
        Pm[i + 1, i] = 1.0
    PT1 = Pm.T
    PT = np.zeros((P, P), np.float32)
    for h in range(2):
        PT[h * DH:(h + 1) * DH, h * DH:(h + 1) * DH] = PT1
    PT = PT.astype(BF16)
    # causal masks for diagonal kb of each q-chunk
    tri = np.zeros((cfg.kb_per_qc, P, cfg.qc_w), np.float32)
    qpos = np.arange(cfg.qc_w)
    for j in range(cfg.kb_per_qc):
        kpos = j * P + np.arange(P)
        tri[j] = (kpos[:, None] <= qpos[None, :]).astype(np.float32)
    tri = tri.astype(BF16)

    scale_q = DH ** -0.5
    # qkv weights, shared across cores: cols regrouped dest-major
    wqkv_l = []
    for l in range(cfg.depth):
        g = attn_g[l] * sD
        wqkv3 = w_qkv[l].reshape(D, 3, H * DH)
        wqf = (g[:, None] * wqkv3[:, 0, :]) * scale_q    # [D, H*DH]
        wkf = g[:, None] * wqkv3[:, 1, :]
        wvf = g[:, None] * wqkv3[:, 2, :]
        blocks = []
        for c in range(NC):
            cs = slice(c * cfg.hd, (c + 1) * cfg.hd)
            blocks += [wqf[:, cs], wkf[:, cs], wvf[:, cs]]
        wqkv_l.append(np.concatenate(blocks, axis=1))    # [D, 3*H*DH]
    wqkv_all = np.stack(wqkv_l, 0).astype(BF16)

    wo_l, w1_l, w2_l, b1_l, b2_l = [], [], [], [], []
    for l in range(cfg.depth):
        wo_l.append(w_out[l])
        gf = ff_g[l] * sD
        w1_l.append(gf[:, None] * w_ff1[l])
        w2_l.append(w_ff2[l])
        b1_l.append(b_ff1[l].reshape(cfg.fb, P).T)
        b2_l.append(b_ff2[l].reshape(cfg.db, P).T)
    wlogg = (fin_g * sD)[:, None] * w_log

    shared = {
        "emb_tab": emb,
        "wqkv": wqkv_all,
        "wout": np.stack(wo_l, 0).astype(BF16),
        "wff1": np.stack(w1_l, 0).astype(BF16),
        "bff1": np.stack(b1_l, 0).astype(np.float32),
        "wff2": np.stack(w2_l, 0).astype(BF16),
        "bff2": np.stack(b2_l, 0).astype(np.float32),
        "wlog": wlogg.astype(BF16),
        "rotPT": PT, "trimask": tri,
    }
    in_maps = []
    for c in range(NC):
        d = dict(shared)
        d["emb_idx"] = x[c * t:(c + 1) * t].reshape(t, 1)
        d["cosq"] = np.ascontiguousarray(cos2[:, c * t:(c + 1) * t])
        d["sinq"] = np.ascontiguousarray(sin2[:, c * t:(c + 1) * t])
        in_maps.append(d)
    return in_maps


_CACHED = {}


def kernel(**inputs):
    import jax
    from jax.sharding import Mesh, PartitionSpec
    from jax.experimental.shard_map import shard_map
    from concourse.bass2jax import (_bass_exec_p, install_neuronx_cc_hook,
                                    partition_id_tensor)
    cfg = FULL
    in_maps = prep_inputs(inputs, cfg)
    if "nc" not in _CACHED:
        _CACHED["nc"] = build_kernel(cfg)
    nc = _CACHED["nc"]
    install_neuronx_cc_hook()
    partition_name = nc.partition_id_tensor.name if nc.partition_id_tensor else None
    in_names, out_names, out_avals = [], [], []
    for alloc in nc.m.functions[0].allocations:
        if not isinstance(alloc, mybir.MemoryLocationSet):
            continue
        name = alloc.memorylocations[0].name
        if alloc.kind == "ExternalInput":
            if name != partition_name:
                in_names.append(name)
        elif alloc.kind == "ExternalOutput":
            out_names.append(name)
            out_avals.append(jax.core.ShapedArray(
                tuple(alloc.tensor_shape), mybir.dt.np(alloc.dtype)))
    all_in = list(in_names) + list(out_names)
    if partition_name is not None:
        all_in.append(partition_name)
    n_params = len(in_names)

    def _body(*args):
        operands = list(args)
        if partition_name is not None:
            operands.append(partition_id_tensor())
        return tuple(_bass_exec_p.bind(
            *operands, out_avals=tuple(out_avals), in_names=tuple(all_in),
            out_names=tuple(out_names), lowering_input_output_aliases=(),
            sim_require_finite=True, sim_require_nnan=True, nc=nc))

    devices = jax.devices()[:NC]
    mesh = Mesh(np.asarray(devices), ("core",))
    n_outs = len(out_names)
    sharded = jax.jit(
        shard_map(_body, mesh=mesh,
                  in_specs=(PartitionSpec("core"),) * (n_params + n_outs),
                  out_specs=(PartitionSpec("core"),) * n_outs,
                  check_rep=False),
        donate_argnums=tuple(range(n_params, n_params + n_outs)),
        keep_unused=True)
    concat_in = [np.concatenate([np.asarray(in_maps[c][nm]) for c in range(NC)], 0)
                 for nm in in_names]
    zeros = [np.zeros((NC * a.shape[0], *a.shape[1:]), a.dtype) for a in out_avals]
    out = sharded(*concat_in, *zeros)
    logits = np.asarray(out[out_names.index("logits_out")])
    return logits.reshape(B, cfg.n, cfg.v).astype(np.float32)


# revision 13
# speedup vs baseline: 1.1986x; 1.0070x over previous
"""Trainium2 Bass kernel for nn_Decoder (4-layer dense transformer decoder).

Sharding (8 NeuronCores), v2:
  - Sequence-parallel residual stream: core c owns tokens [256c, 256c+256).
  - qkv is computed token-locally for ALL heads (full qkv weights are
    replicated, streamed bf16), with rope applied locally; a single
    AllToAll (1.5MiB out) redistributes q/k/v to head-parallel form
    (2 heads/core, all 2048 tokens). This replaces the v1 per-layer 4MiB
    AllGather (2.2x cheaper collective).
  - Attention is head-parallel; the softmax denominator rides the A@V
    matmul as a ones-column appended to V (no separate ones-matmul chain).
  - Attention outputs return to token-local form via a small AllToAll
    (0.5MiB); out-proj/FFN/logits run token-locally with replicated
    weights streamed from HBM in large contiguous chunks (>=1KB elements
    to dodge the sub-512B DMA read-modify-write penalty).
  - Gamma, sqrt(D) and the attention 1/sqrt(dh) scale are folded into
    weights on the host; biases ride the ACT activation ops.

Layout: everything on-chip is transposed — [D(partitions), tokens(free)].
"""
import numpy as np
import ml_dtypes

import concourse.bass as bass
import concourse.mybir as mybir
import concourse.tile as tile
from concourse import bacc
from concourse.tile import TileContext
from concourse.masks import make_identity

BF16 = np.dtype(ml_dtypes.bfloat16)
AF = mybir.ActivationFunctionType
P = 128

# model dims
V, D, DEPTH, H, DH, FF = 32000, 1024, 4, 16, 64, 4096
B, N = 1, 2048
NC = 8  # cores


class Cfg:
    def __init__(self, n=N, depth=DEPTH, v=V, ff=FF):
        self.n = n            # total tokens
        self.depth = depth
        self.v = v
        self.ff = ff
        self.t = n // NC      # tokens per core
        self.tb = self.t // P           # token blocks per core
        self.db = D // P                # D blocks (8)
        self.fb = ff // P               # FF blocks (32)
        self.heads_per_core = H // NC   # 2
        self.hd = self.heads_per_core * DH  # 128 head-dims per core
        self.n_kb = n // P              # key blocks (16)
        self.qc_w = min(512, n)         # query chunk width
        self.n_qc = n // self.qc_w      # query chunks
        self.kb_per_qc = self.qc_w // P  # 4


FULL = Cfg()


def build_kernel(cfg=FULL):
    n, t, depth, v, ff = cfg.n, cfg.t, cfg.depth, cfg.v, cfg.ff
    db, fb, tb = cfg.db, cfg.fb, cfg.tb
    hd = cfg.hd
    nqb = 3 * NC          # qkv partition blocks (24): dest-major [q,k,v]
    f32, bf16, i32 = mybir.dt.float32, mybir.dt.bfloat16, mybir.dt.int32

    nc = bacc.Bacc(None, target_bir_lowering=False, debug=False, num_devices=NC)

    # ---------- DRAM I/O ----------
    emb_idx = nc.dram_tensor("emb_idx", [t, 1], i32, kind="ExternalInput")
    emb_tab = nc.dram_tensor("emb_tab", [v, D], f32, kind="ExternalInput")
    # full qkv weights, columns regrouped dest-major: block 3c+j (j=q,k,v)
    wqkv = nc.dram_tensor("wqkv", [depth, D, nqb * P], bf16, kind="ExternalInput")
    wout = nc.dram_tensor("wout", [depth, D, D], bf16, kind="ExternalInput")
    wff1 = nc.dram_tensor("wff1", [depth, D, ff], bf16, kind="ExternalInput")
    bff1 = nc.dram_tensor("bff1", [depth, P, fb], f32, kind="ExternalInput")
    wff2 = nc.dram_tensor("wff2", [depth, ff, D], bf16, kind="ExternalInput")
    bff2 = nc.dram_tensor("bff2", [depth, P, db], f32, kind="ExternalInput")
    wlog = nc.dram_tensor("wlog", [D, v], bf16, kind="ExternalInput")
    cosq = nc.dram_tensor("cosq", [P, t], bf16, kind="ExternalInput")
    sinq = nc.dram_tensor("sinq", [P, t], bf16, kind="ExternalInput")
    rotPT = nc.dram_tensor("rotPT", [P, P], bf16, kind="ExternalInput")
    # diagonal causal masks for the 512-wide q-chunk: j = kb offset in chunk
    trimask = nc.dram_tensor("trimask", [cfg.kb_per_qc, P, cfg.qc_w], bf16,
                             kind="ExternalInput")
    logits_out = nc.dram_tensor("logits_out", [t, v], f32, kind="ExternalOutput")

    # collective bounce buffers (reused across layers)
    a2a_in = nc.dram_tensor("a2a_in", [NC, 3 * P, t], bf16)
    a2a_out = nc.dram_tensor("a2a_out", [NC, 3 * P, t], bf16)
    a2o_in = nc.dram_tensor("a2o_in", [NC, hd, t], bf16)
    a2o_out = nc.dram_tensor("a2o_out", [NC, hd, t], bf16)

    with TileContext(nc) as tc:
        with tc.tile_pool(name="const", bufs=1) as cpool, \
             tc.tile_pool(name="resid", bufs=1) as rpool, \
             tc.tile_pool(name="work", bufs=1) as wpool, \
             tc.tile_pool(name="wts", bufs=2) as wtpool, \
             tc.tile_pool(name="qkvw", bufs=2) as qwpool, \
             tc.tile_pool(name="small", bufs=3) as spool, \
             tc.tile_pool(name="attn", bufs=1) as apool, \
             tc.tile_pool(name="pexp", bufs=3) as epool, \
             tc.tile_pool(name="psum_acc", bufs=1, space="PSUM") as pacc, \
             tc.tile_pool(name="psum_s", bufs=4, space="PSUM") as ps:

            # ---------- constants ----------
            ident = cpool.tile([P, P], f32)
            make_identity(nc, ident[:])
            ident_bf = cpool.tile([P, P], bf16)
            nc.vector.tensor_copy(ident_bf[:], ident[:])
            ones_bf = cpool.tile([P, 1], bf16)
            nc.vector.memset(ones_bf[:], 1.0)
            ones_row = cpool.tile([1, P], f32)
            nc.vector.memset(ones_row[:], 1.0)
            cos_t = cpool.tile([P, t], bf16)
            sin_t = cpool.tile([P, t], bf16)
            rot_t = cpool.tile([P, P], bf16)
            nc.sync.dma_start(cos_t[:], cosq[:, :])
            nc.sync.dma_start(sin_t[:], sinq[:, :])
            nc.sync.dma_start(rot_t[:], rotPT[:, :])
            mask_t = cpool.tile([P, cfg.kb_per_qc, cfg.qc_w], bf16)
            nc.sync.dma_start(
                mask_t[:], trimask[:, :, :].rearrange("j p q -> p j q"))
            # V tiles (per head): col 64 stays all-ones for the softmax
            # denominator; cols 0:64 rewritten each layer.
            vh = []
            for h in range(cfg.heads_per_core):
                vt = cpool.tile([P, cfg.n_kb, DH + 1], bf16, name=f"vh{h}")
                nc.vector.memset(vt[:], 1.0)
                vh.append(vt)

            # ---------- embedding gather -> hT [P, db, t] f32 ----------
            hT = rpool.tile([P, db, t], f32)
            for tbi in range(tb):
                idx_t = spool.tile([P, 1], i32, tag="idx")
                nc.sync.dma_start(idx_t[:], emb_idx[tbi * P:(tbi + 1) * P, :])
                g_t = wpool.tile([P, D], f32, tag="gather")
                nc.gpsimd.indirect_dma_start(
                    out=g_t[:], out_offset=None, in_=emb_tab[:, :],
                    in_offset=bass.IndirectOffsetOnAxis(ap=idx_t[:, :1], axis=0))
                for dbi in range(db):
                    ptr = ps.tile([P, P], f32, tag="ps")
                    nc.tensor.transpose(ptr[:], g_t[:, dbi * P:(dbi + 1) * P],
                                        ident[:])
                    nc.any.tensor_copy(
                        hT[:, dbi, tbi * P:(tbi + 1) * P], ptr[:])

            def rms_norm_cast(src_f32, dst_bf):
                """dst_bf[P, db, t] = src * rsqrt(sum_D(src^2)); sqrt(D)*gamma
                is folded into the consuming weights."""
                sq = wpool.tile([P, db, t], bf16, tag="normsq")
                for dbi in range(db):
                    nc.vector.tensor_tensor(
                        sq[:, dbi, :], src_f32[:, dbi, :], src_f32[:, dbi, :],
                        mybir.AluOpType.mult)
                psum_n = pacc.tile([1, t], f32, tag="acc_a")
                for dbi in range(db):
                    nc.tensor.matmul(psum_n[:], ones_bf[:], sq[:, dbi, :],
                                     start=(dbi == 0), stop=(dbi == db - 1))
                rt = spool.tile([1, t], f32, tag="norm_rt")
                nc.scalar.activation(rt[:], psum_n[:], AF.Sqrt)
                inv = spool.tile([1, t], f32, tag="norm_inv")
                nc.vector.reciprocal(inv[:], rt[:])
                psum_b = ps.tile([P, t], f32, tag="ps")
                nc.tensor.matmul(psum_b[:], ones_row[:], inv[:],
                                 start=True, stop=True)
                invb = spool.tile([P, t], f32, tag="norm_invb")
                nc.vector.tensor_copy(invb[:], psum_b[:])
                for dbi in range(db):
                    nc.vector.tensor_tensor(
                        dst_bf[:, dbi, :], src_f32[:, dbi, :], invb[:],
                        mybir.AluOpType.mult)

            # ================= layers =================
            for l in range(depth):
                # ----- norm1 -----
                xn = wpool.tile([P, db, t], bf16, tag="xn")
                rms_norm_cast(hT, xn)

                # ----- token-local qkv (all heads) + rope, then AllToAll -----
                qkv_sb = wpool.tile([P, nqb, t], bf16, tag="qkv_sb")
                QW = nqb * P // 4     # qkv weight cols per load group (768)
                for quarter in range(4):
                    wq = qwpool.tile([P, db, QW], bf16, tag="wq")
                    nc.sync.dma_start(
                        wq[:], wqkv[l, :, quarter * QW:
                                     (quarter + 1) * QW].rearrange(
                            "(o p) c -> p o c", p=P))
                    for blk in range(nqb // 4):
                        b = quarter * (nqb // 4) + blk
                        comp = b // NC        # 0=q, 1=k, 2=v (comp-major)
                        pq = ps.tile([P, t], f32, tag="ps")
                        for dbi in range(db):
                            nc.tensor.matmul(
                                pq[:], wq[:, dbi, blk * P:(blk + 1) * P],
                                xn[:, dbi, :],
                                start=(dbi == 0), stop=(dbi == db - 1))
                        if comp == 2:
                            nc.vector.tensor_copy(qkv_sb[:, b, :], pq[:])
                        else:
                            raw = epool.tile([P, t], bf16, tag="rope_raw")
                            nc.vector.tensor_copy(raw[:], pq[:])
                            prot = ps.tile([P, t], f32, tag="ps")
                            nc.tensor.matmul(prot[:], rot_t[:], raw[:],
                                             start=True, stop=True)
                            t1 = epool.tile([P, t], bf16, tag="rope_t1")
                            nc.vector.tensor_tensor(
                                t1[:], prot[:], sin_t[:],
                                mybir.AluOpType.mult)
                            t2 = epool.tile([P, t], bf16, tag="rope_t2")
                            nc.vector.tensor_tensor(
                                t2[:], raw[:], cos_t[:],
                                mybir.AluOpType.mult)
                            nc.vector.tensor_tensor(
                                qkv_sb[:, b, :], t1[:], t2[:],
                                mybir.AluOpType.add)
                for j in range(3):
                    nc.sync.dma_start(
                        a2a_in[:, j * P:(j + 1) * P, :]
                        .rearrange("c p t -> p c t"),
                        qkv_sb[:, j * NC:(j + 1) * NC, :])
                nc.gpsimd.collective_compute(
                    "AllToAll", mybir.AluOpType.bypass,
                    replica_groups=[list(range(NC))],
                    ins=[a2a_in.ap().opt()], outs=[a2a_out.ap().opt()])

                # ----- unpack: qT/kT [P, n]; V -> per-head [tok, dh|ones] -----
                qT = apool.tile([P, n], bf16, tag="qT")
                kT = apool.tile([P, n], bf16, tag="kT")
                vT = apool.tile([P, n], bf16, tag="vT")
                for dst, j in ((qT, 0), (kT, 1), (vT, 2)):
                    nc.sync.dma_start(
                        dst[:].rearrange("d (c t) -> d c t", c=NC),
                        a2a_out[:, j * P:(j + 1) * P, :]
                        .rearrange("c d t -> d c t"))
                for kbi in range(cfg.n_kb):
                    ptv = ps.tile([P, P], f32, tag="ps")
                    nc.tensor.matmul(
                        ptv[:], vT[:, kbi * P:(kbi + 1) * P], ident_bf[:],
                        start=True, stop=True)
                    for h in range(cfg.heads_per_core):
                        nc.vector.tensor_copy(
                            vh[h][:, kbi, :DH], ptv[:, h * DH:(h + 1) * DH])

                # ----- attention (2 heads), output attT [P, n] bf16 -----
                attT = apool.tile([P, n], bf16, tag="attT")
                for h in range(cfg.heads_per_core):
                    hsl = slice(h * DH, (h + 1) * DH)
                    for qc_i in range(cfg.n_qc):
                        qsl = slice(qc_i * cfg.qc_w, (qc_i + 1) * cfg.qc_w)
                        n_kb_q = (qc_i + 1) * cfg.kb_per_qc
                        pav = pacc.tile([DH + 1, cfg.qc_w], f32, tag="acc_a")
                        for kbi in range(n_kb_q):
                            pscr = ps.tile([P, cfg.qc_w], f32, tag="ps")
                            nc.tensor.matmul(
                                pscr[:], kT[hsl, kbi * P:(kbi + 1) * P],
                                qT[hsl, qsl], start=True, stop=True)
                            pe = epool.tile([P, cfg.qc_w], bf16, tag="att_exp")
                            nc.scalar.activation(pe[:], pscr[:], AF.Exp)
                            j = kbi - qc_i * cfg.kb_per_qc
                            if j >= 0:
                                nc.vector.tensor_tensor(
                                    pe[:], pe[:], mask_t[:, j, :],
                                    mybir.AluOpType.mult)
                            first, last = kbi == 0, kbi == n_kb_q - 1
                            nc.tensor.matmul(pav[:], vh[h][:, kbi, :], pe[:],
                                             start=first, stop=last)
                        inv = spool.tile([1, cfg.qc_w], f32, tag="att_inv")
                        nc.vector.reciprocal(inv[:], pav[DH:DH + 1, :])
                        pb = ps.tile([DH, cfg.qc_w], f32, tag="ps")
                        nc.tensor.matmul(pb[:], ones_row[:, :DH], inv[:],
                                         start=True, stop=True)
                        invb = spool.tile([DH, cfg.qc_w], f32, tag="att_invb")
                        nc.vector.tensor_copy(invb[:], pb[:])
                        nc.vector.tensor_tensor(
                            attT[hsl, qsl], pav[:DH, :], invb[:],
                            mybir.AluOpType.mult)

                # ----- AllToAll back to token-local -----
                nc.sync.dma_start(
                    a2o_in[:, :, :].rearrange("c d t -> d c t"),
                    attT[:].rearrange("d (c t) -> d c t", c=NC))
                nc.gpsimd.collective_compute(
                    "AllToAll", mybir.AluOpType.bypass,
                    replica_groups=[list(range(NC))],
                    ins=[a2o_in.ap().opt()], outs=[a2o_out.ap().opt()])
                attC = wpool.tile([P, NC, t], bf16, tag="attC")
                nc.sync.dma_start(
                    attC[:], a2o_out[:, :, :].rearrange("c d t -> d c t"))

                # ----- out-proj + residual -----
                # contraction split in two halves of 4 hb blocks so the
                # weight tile double-buffers at half size; 8 outputs live in
                # 4 pair-packed psum accumulators across both halves
                pgo = [pacc.tile([P, 2, t], f32, tag=f"acc_{'abcd'[pi]}",
                                 name=f"oacc_{l}_{pi}") for pi in range(4)]
                for half in range(2):
                    wo = wtpool.tile([P, NC // 2, D], bf16, tag="wo")
                    nc.sync.dma_start(
                        wo[:], wout[l, half * (D // 2):(half + 1) * (D // 2),
                                    :].rearrange("(hb p) q -> p hb q", p=P))
                    for dci in range(db):
                        for hb in range(NC // 2):
                            nc.tensor.matmul(
                                pgo[dci // 2][:, dci % 2, :],
                                wo[:, hb, dci * P:(dci + 1) * P],
                                attC[:, half * 4 + hb, :],
                                start=(half == 0 and hb == 0),
                                stop=(half == 1 and hb == NC // 2 - 1))
                for dci in range(db):
                    nc.vector.tensor_tensor(hT[:, dci, :], hT[:, dci, :],
                                            pgo[dci // 2][:, dci % 2, :],
                                            mybir.AluOpType.add)

                # ----- norm2 + FFN (token-local, no collective) -----
                xn2 = wpool.tile([P, db, t], bf16, tag="xn")
                rms_norm_cast(hT, xn2)
                b1 = spool.tile([P, fb], f32, tag="b1")
                nc.sync.dma_start(b1[:], bff1[l, :, :])
                b2 = spool.tile([P, db], f32, tag="b2")
                nc.sync.dma_start(b2[:], bff2[l, :, :])
                # fused ff1 -> gelu -> ff2: 4 pair-psum accumulators hold the
                # 8 D-chunk outputs; act chunk is transient. Weights stream in
                # 512-ff-col groups (contiguous >=1KB DMA elements).
                pgs = []
                for pi in range(4):
                    pg_i = pacc.tile([P, 2, t], f32, tag=f"acc_{'abcd'[pi]}",
                                     name=f"ffacc_{l}_{pi}")
                    pgs.append(pg_i)
                GW = 512            # ff cols per weight-load group
                ng = ff // GW
                fpg = GW // P       # fci per group (4)
                for gi in range(ng):
                    w1g = wtpool.tile([P, db, GW], bf16, tag="w1g")
                    nc.sync.dma_start(
                        w1g[:], wff1[l, :, gi * GW:(gi + 1) * GW].rearrange(
                            "(o p) c -> p o c", p=P))
                    w2g = wtpool.tile([P, fpg, db, P], bf16, tag="w2g")
                    for fi in range(fpg):
                        nc.sync.dma_start(
                            w2g[:, fi, :, :],
                            wff2[l, (gi * fpg + fi) * P:
                                 (gi * fpg + fi + 1) * P, :].rearrange(
                                "p (dc q) -> p dc q", q=P))
                    for fi in range(fpg):
                        fci = gi * fpg + fi
                        pf = ps.tile([P, t], f32, tag="ps")
                        for dbi in range(db):
                            nc.tensor.matmul(
                                pf[:], w1g[:, dbi, fi * P:(fi + 1) * P],
                                xn2[:, dbi, :],
                                start=(dbi == 0), stop=(dbi == db - 1))
                        act_c = epool.tile([P, t], bf16, tag="act_c")
                        nc.scalar.activation(act_c[:], pf[:], AF.Gelu,
                                             bias=b1[:, fci:fci + 1])
                        for dci in range(db):
                            nc.tensor.matmul(
                                pgs[dci // 2][:, dci % 2, :],
                                w2g[:, fi, dci, :], act_c[:],
                                start=(fci == 0 and dci % 2 == 0),
                                stop=(fci == fb - 1 and dci % 2 == 1))
                for dci in range(db):
                    tmp = spool.tile([P, t], f32, tag="ff2_tmp")
                    nc.scalar.activation(tmp[:], pgs[dci // 2][:, dci % 2, :],
                                         AF.Identity, bias=b2[:, dci:dci + 1])
                    nc.vector.tensor_tensor(hT[:, dci, :], hT[:, dci, :],
                                            tmp[:], mybir.AluOpType.add)

            # ================= final norm + logits =================
            xnf = wpool.tile([P, db, t], bf16, tag="xn")
            rms_norm_cast(hT, xnf)
            vchunks = []
            off = 0
            while off < v:
                w = min(512, v - off)
                vchunks.append((off, w))
                off += w
            for (off, w) in vchunks:
                wl = wtpool.tile([P, db, 512], bf16, tag="wl")
                nc.sync.dma_start(
                    wl[:, :, :w],
                    wlog[:, off:off + w].rearrange("(o p) c -> p o c", p=P))
                for tbi in range(tb):
                    pl = pacc.tile([P, 512], f32, tag=f"acc_{'ab'[tbi % 2]}",
                                   name=f"pl_{off}_{tbi}")
                    for dbi in range(db):
                        nc.tensor.matmul(
                            pl[:, :w], xnf[:, dbi, tbi * P:(tbi + 1) * P],
                            wl[:, dbi, :w],
                            start=(dbi == 0), stop=(dbi == db - 1))
                    ot = spool.tile([P, 512], f32, tag="log_out")
                    nc.vector.tensor_copy(ot[:, :w], pl[:, :w])
                    nc.sync.dma_start(
                        logits_out[tbi * P:(tbi + 1) * P, off:off + w],
                        ot[:, :w])
    nc.finalize()
    return nc


# ======================= host side =======================

def prep_inputs(inputs, cfg=FULL):
    """Full model inputs -> list of 8 per-core input dicts (numpy)."""
    n, t, depth, v, ff = cfg.n, cfg.t, cfg.depth, cfg.v, cfg.ff
    x = np.asarray(inputs["x"]).reshape(-1)[:n].astype(np.int32)
    emb = np.asarray(inputs["token_emb"], dtype=np.float32)[:v]
    attn_g = np.asarray(inputs["attn_gamma"], dtype=np.float32)
    w_qkv = np.asarray(inputs["w_qkv"], dtype=np.float32)
    w_out = np.asarray(inputs["w_attn_out"], dtype=np.float32)
    ff_g = np.asarray(inputs["ff_gamma"], dtype=np.float32)
    w_ff1 = np.asarray(inputs["w_ff1"], dtype=np.float32)[:, :, :ff]
    b_ff1 = np.asarray(inputs["b_ff1"], dtype=np.float32)[:, :ff]
    w_ff2 = np.asarray(inputs["w_ff2"], dtype=np.float32)[:, :ff, :]
    b_ff2 = np.asarray(inputs["b_ff2"], dtype=np.float32)
    fin_g = np.asarray(inputs["final_gamma"], dtype=np.float32)
    w_log = np.asarray(inputs["w_logits"], dtype=np.float32)[:, :v]

    sD = float(np.sqrt(D))
    # rope tables
    inv_freq = 1.0 / (10000.0 ** (np.arange(0, DH, 2, dtype=np.float32) / DH))
    freqs = np.arange(n, dtype=np.float32)[:, None] * inv_freq[None, :]
    freqs = np.repeat(freqs, 2, axis=-1)          # [n, DH]
    cos = np.cos(freqs).T                          # [DH, n]
    sin = np.sin(freqs).T
    cos2 = np.tile(cos, (2, 1)).astype(BF16)       # [128, n]
    sin2 = np.tile(sin, (2, 1)).astype(BF16)
    # rot matrix PT st rot(x) = PT.T @ x, block-diag per head (DH x DH)
    Pm = np.zeros((DH, DH), np.float32)
    for i in range(0, DH, 2):
        Pm[i, i + 1] = -1.0
        Pm[i + 1, i] = 1.0
    PT1 = Pm.T
    PT = np.zeros((P, P), np.float32)
    for h in range(2):
        PT[h * DH:(h + 1) * DH, h * DH:(h + 1) * DH] = PT1
    PT = PT.astype(BF16)
    # causal masks for diagonal kb of each q-chunk
    tri = np.zeros((cfg.kb_per_qc, P, cfg.qc_w), np.float32)
    qpos = np.arange(cfg.qc_w)
    for j in range(cfg.kb_per_qc):
        kpos = j * P + np.arange(P)
        tri[j] = (kpos[:, None] <= qpos[None, :]).astype(np.float32)
    tri = tri.astype(BF16)

    scale_q = DH ** -0.5
    # qkv weights, shared across cores: cols regrouped dest-major
    wqkv_l = []
    for l in range(cfg.depth):
        g = attn_g[l] * sD
        wqkv3 = w_qkv[l].reshape(D, 3, H * DH)
        wqf = (g[:, None] * wqkv3[:, 0, :]) * scale_q    # [D, H*DH]
        wkf = g[:, None] * wqkv3[:, 1, :]
        wvf = g[:, None] * wqkv3[:, 2, :]
        # component-major: cols = [q all heads | k all heads | v all heads]
        wqkv_l.append(np.concatenate([wqf, wkf, wvf], axis=1))  # [D, 3*H*DH]
    wqkv_all = np.stack(wqkv_l, 0).astype(BF16)

    wo_l, w1_l, w2_l, b1_l, b2_l = [], [], [], [], []
    for l in range(cfg.depth):
        wo_l.append(w_out[l])
        gf = ff_g[l] * sD
        w1_l.append(gf[:, None] * w_ff1[l])
        w2_l.append(w_ff2[l])
        b1_l.append(b_ff1[l].reshape(cfg.fb, P).T)
        b2_l.append(b_ff2[l].reshape(cfg.db, P).T)
    wlogg = (fin_g * sD)[:, None] * w_log

    shared = {
        "emb_tab": emb,
        "wqkv": wqkv_all,
        "wout": np.stack(wo_l, 0).astype(BF16),
        "wff1": np.stack(w1_l, 0).astype(BF16),
        "bff1": np.stack(b1_l, 0).astype(np.float32),
        "wff2": np.stack(w2_l, 0).astype(BF16),
        "bff2": np.stack(b2_l, 0).astype(np.float32),
        "wlog": wlogg.astype(BF16),
        "rotPT": PT, "trimask": tri,
    }
    in_maps = []
    for c in range(NC):
        d = dict(shared)
        d["emb_idx"] = x[c * t:(c + 1) * t].reshape(t, 1)
        d["cosq"] = np.ascontiguousarray(cos2[:, c * t:(c + 1) * t])
        d["sinq"] = np.ascontiguousarray(sin2[:, c * t:(c + 1) * t])
        in_maps.append(d)
    return in_maps


_CACHED = {}


def kernel(**inputs):
    import jax
    from jax.sharding import Mesh, PartitionSpec
    from jax.experimental.shard_map import shard_map
    from concourse.bass2jax import (_bass_exec_p, install_neuronx_cc_hook,
                                    partition_id_tensor)
    cfg = FULL
    in_maps = prep_inputs(inputs, cfg)
    if "nc" not in _CACHED:
        _CACHED["nc"] = build_kernel(cfg)
    nc = _CACHED["nc"]
    install_neuronx_cc_hook()
    partition_name = nc.partition_id_tensor.name if nc.partition_id_tensor else None
    in_names, out_names, out_avals = [], [], []
    for alloc in nc.m.functions[0].allocations:
        if not isinstance(alloc, mybir.MemoryLocationSet):
            continue
        name = alloc.memorylocations[0].name
        if alloc.kind == "ExternalInput":
            if name != partition_name:
                in_names.append(name)
        elif alloc.kind == "ExternalOutput":
            out_names.append(name)
            out_avals.append(jax.core.ShapedArray(
                tuple(alloc.tensor_shape), mybir.dt.np(alloc.dtype)))
    all_in = list(in_names) + list(out_names)
    if partition_name is not None:
        all_in.append(partition_name)
    n_params = len(in_names)

    def _body(*args):
        operands = list(args)
        if partition_name is not None:
            operands.append(partition_id_tensor())
        return tuple(_bass_exec_p.bind(
            *operands, out_avals=tuple(out_avals), in_names=tuple(all_in),
            out_names=tuple(out_names), lowering_input_output_aliases=(),
            sim_require_finite=True, sim_require_nnan=True, nc=nc))

    devices = jax.devices()[:NC]
    mesh = Mesh(np.asarray(devices), ("core",))
    n_outs = len(out_names)
    sharded = jax.jit(
        shard_map(_body, mesh=mesh,
                  in_specs=(PartitionSpec("core"),) * (n_params + n_outs),
                  out_specs=(PartitionSpec("core"),) * n_outs,
                  check_rep=False),
        donate_argnums=tuple(range(n_params, n_params + n_outs)),
        keep_unused=True)
    concat_in = [np.concatenate([np.asarray(in_maps[c][nm]) for c in range(NC)], 0)
                 for nm in in_names]
    zeros = [np.zeros((NC * a.shape[0], *a.shape[1:]), a.dtype) for a in out_avals]
    out = sharded(*concat_in, *zeros)
    logits = np.asarray(out[out_names.index("logits_out")])
    return logits.reshape(B, cfg.n, cfg.v).astype(np.float32)


# revision 17
# speedup vs baseline: 1.2378x; 1.0328x over previous
"""Trainium2 Bass kernel for nn_Decoder (4-layer dense transformer decoder).

Sharding (8 NeuronCores), v2:
  - Sequence-parallel residual stream: core c owns tokens [256c, 256c+256).
  - qkv is computed token-locally for ALL heads (full qkv weights are
    replicated, streamed bf16), with rope applied locally; a single
    AllToAll (1.5MiB out) redistributes q/k/v to head-parallel form
    (2 heads/core, all 2048 tokens). This replaces the v1 per-layer 4MiB
    AllGather (2.2x cheaper collective).
  - Attention is head-parallel; the softmax denominator rides the A@V
    matmul as a ones-column appended to V (no separate ones-matmul chain).
  - Attention outputs return to token-local form via a small AllToAll
    (0.5MiB); out-proj/FFN/logits run token-locally with replicated
    weights streamed from HBM in large contiguous chunks (>=1KB elements
    to dodge the sub-512B DMA read-modify-write penalty).
  - Gamma, sqrt(D) and the attention 1/sqrt(dh) scale are folded into
    weights on the host; biases ride the ACT activation ops.

Layout: everything on-chip is transposed — [D(partitions), tokens(free)].
"""
import numpy as np
import ml_dtypes

import concourse.bass as bass
import concourse.mybir as mybir
import concourse.tile as tile
from concourse import bacc
from concourse.tile import TileContext
from concourse.masks import make_identity

BF16 = np.dtype(ml_dtypes.bfloat16)
AF = mybir.ActivationFunctionType
P = 128

# model dims
V, D, DEPTH, H, DH, FF = 32000, 1024, 4, 16, 64, 4096
B, N = 1, 2048
NC = 8  # cores


class Cfg:
    def __init__(self, n=N, depth=DEPTH, v=V, ff=FF):
        self.n = n            # total tokens
        self.depth = depth
        self.v = v
        self.ff = ff
        self.t = n // NC      # tokens per core
        self.tb = self.t // P           # token blocks per core
        self.db = D // P                # D blocks (8)
        self.fb = ff // P               # FF blocks (32)
        self.heads_per_core = H // NC   # 2
        self.hd = self.heads_per_core * DH  # 128 head-dims per core
        self.n_kb = n // P              # key blocks (16)
        self.qc_w = min(512, n)         # query chunk width
        self.n_qc = n // self.qc_w      # query chunks
        self.kb_per_qc = self.qc_w // P  # 4


FULL = Cfg()


def build_kernel(cfg=FULL):
    n, t, depth, v, ff = cfg.n, cfg.t, cfg.depth, cfg.v, cfg.ff
    db, fb, tb = cfg.db, cfg.fb, cfg.tb
    hd = cfg.hd
    nqb = 3 * NC          # qkv partition blocks (24): dest-major [q,k,v]
    f32, bf16, i32 = mybir.dt.float32, mybir.dt.bfloat16, mybir.dt.int32

    nc = bacc.Bacc(None, target_bir_lowering=False, debug=False, num_devices=NC)

    # ---------- DRAM I/O ----------
    emb_idx = nc.dram_tensor("emb_idx", [t, 1], i32, kind="ExternalInput")
    emb_tab = nc.dram_tensor("emb_tab", [v, D], f32, kind="ExternalInput")
    # full qkv weights, columns regrouped dest-major: block 3c+j (j=q,k,v)
    wqkv = nc.dram_tensor("wqkv", [depth, D, nqb * P], bf16, kind="ExternalInput")
    wout = nc.dram_tensor("wout", [depth, D, D], bf16, kind="ExternalInput")
    wff1 = nc.dram_tensor("wff1", [depth, D, ff], bf16, kind="ExternalInput")
    bff1 = nc.dram_tensor("bff1", [depth, P, fb], f32, kind="ExternalInput")
    wff2 = nc.dram_tensor("wff2", [depth, ff, D], bf16, kind="ExternalInput")
    bff2 = nc.dram_tensor("bff2", [depth, P, db], f32, kind="ExternalInput")
    wlog = nc.dram_tensor("wlog", [D, v], bf16, kind="ExternalInput")
    cosq = nc.dram_tensor("cosq", [P, t], bf16, kind="ExternalInput")
    sinq = nc.dram_tensor("sinq", [P, t], bf16, kind="ExternalInput")
    rotPT = nc.dram_tensor("rotPT", [P, P], bf16, kind="ExternalInput")
    # diagonal causal masks for the 512-wide q-chunk: j = kb offset in chunk
    trimask = nc.dram_tensor("trimask", [cfg.kb_per_qc, P, cfg.qc_w], bf16,
                             kind="ExternalInput")
    logits_out = nc.dram_tensor("logits_out", [t, v], bf16, kind="ExternalOutput")

    # collective bounce buffers (reused across layers)
    a2a_in = nc.dram_tensor("a2a_in", [NC, 3 * P, t], bf16)
    a2a_out = nc.dram_tensor("a2a_out", [NC, 3 * P, t], bf16)
    a2o_in = nc.dram_tensor("a2o_in", [NC, hd, t], bf16)
    a2o_out = nc.dram_tensor("a2o_out", [NC, hd, t], bf16)

    with TileContext(nc) as tc:
        with tc.tile_pool(name="const", bufs=1) as cpool, \
             tc.tile_pool(name="resid", bufs=1) as rpool, \
             tc.tile_pool(name="work", bufs=1) as wpool, \
             tc.tile_pool(name="wts", bufs=2) as wtpool, \
             tc.tile_pool(name="qkvw", bufs=2) as qwpool, \
             tc.tile_pool(name="small", bufs=3) as spool, \
             tc.tile_pool(name="attn", bufs=1) as apool, \
             tc.tile_pool(name="pexp", bufs=3) as epool, \
             tc.tile_pool(name="psum_acc", bufs=1, space="PSUM") as pacc, \
             tc.tile_pool(name="psum_s", bufs=2, space="PSUM") as ps:

            # ---------- constants ----------
            ident = cpool.tile([P, P], f32)
            make_identity(nc, ident[:])
            ident_bf = cpool.tile([P, P], bf16)
            nc.vector.tensor_copy(ident_bf[:], ident[:])
            ones_bf = cpool.tile([P, 1], bf16)
            nc.vector.memset(ones_bf[:], 1.0)
            ones_row = cpool.tile([1, P], f32)
            nc.vector.memset(ones_row[:], 1.0)
            cos_t = cpool.tile([P, t], bf16)
            sin_t = cpool.tile([P, t], bf16)
            rot_t = cpool.tile([P, P], bf16)
            nc.sync.dma_start(cos_t[:], cosq[:, :])
            nc.sync.dma_start(sin_t[:], sinq[:, :])
            nc.sync.dma_start(rot_t[:], rotPT[:, :])
            mask_t = cpool.tile([P, cfg.kb_per_qc, cfg.qc_w], bf16)
            nc.sync.dma_start(
                mask_t[:], trimask[:, :, :].rearrange("j p q -> p j q"))
            # V tiles (per head): col 64 stays all-ones for the softmax
            # denominator; cols 0:64 rewritten each layer.
            vh = []
            for h in range(cfg.heads_per_core):
                vt = cpool.tile([P, cfg.n_kb, DH + 1], bf16, name=f"vh{h}")
                nc.vector.memset(vt[:], 1.0)
                vh.append(vt)

            # ---------- embedding gather -> hT [P, db, t] f32 ----------
            hT = rpool.tile([P, db, t], f32)
            for tbi in range(tb):
                idx_t = spool.tile([P, 1], i32, tag="idx")
                nc.sync.dma_start(idx_t[:], emb_idx[tbi * P:(tbi + 1) * P, :])
                g_t = wpool.tile([P, D], f32, tag="gather")
                nc.gpsimd.indirect_dma_start(
                    out=g_t[:], out_offset=None, in_=emb_tab[:, :],
                    in_offset=bass.IndirectOffsetOnAxis(ap=idx_t[:, :1], axis=0))
                for dbi in range(db):
                    ptr = ps.tile([P, P], f32, tag="ps")
                    nc.tensor.transpose(ptr[:], g_t[:, dbi * P:(dbi + 1) * P],
                                        ident[:])
                    nc.any.tensor_copy(
                        hT[:, dbi, tbi * P:(tbi + 1) * P], ptr[:])

            def rms_norm_cast(src_f32, dst_bf):
                """dst_bf[P, db, t] = src * rsqrt(sum_D(src^2)); sqrt(D)*gamma
                is folded into the consuming weights."""
                sq = wpool.tile([P, db, t], bf16, tag="normsq")
                for dbi in range(db):
                    nc.vector.tensor_tensor(
                        sq[:, dbi, :], src_f32[:, dbi, :], src_f32[:, dbi, :],
                        mybir.AluOpType.mult)
                psum_n = pacc.tile([1, t], f32, tag="acc_a")
                for dbi in range(db):
                    nc.tensor.matmul(psum_n[:], ones_bf[:], sq[:, dbi, :],
                                     start=(dbi == 0), stop=(dbi == db - 1))
                rt = spool.tile([1, t], f32, tag="norm_rt")
                nc.scalar.activation(rt[:], psum_n[:], AF.Sqrt)
                inv = spool.tile([1, t], f32, tag="norm_inv")
                nc.vector.reciprocal(inv[:], rt[:])
                psum_b = ps.tile([P, t], f32, tag="ps")
                nc.tensor.matmul(psum_b[:], ones_row[:], inv[:],
                                 start=True, stop=True)
                invb = spool.tile([P, t], f32, tag="norm_invb")
                nc.vector.tensor_copy(invb[:], psum_b[:])
                for dbi in range(db):
                    nc.vector.tensor_tensor(
                        dst_bf[:, dbi, :], src_f32[:, dbi, :], invb[:],
                        mybir.AluOpType.mult)

            # ================= layers =================
            for l in range(depth):
                # ----- norm1 -----
                xn = wpool.tile([P, db, t], bf16, tag="xn")
                rms_norm_cast(hT, xn)

                # ----- token-local qkv (all heads) + rope, then AllToAll -----
                qkv_sb = wpool.tile([P, nqb, t], bf16, tag="qkv_sb")
                QW = nqb * P // 4     # qkv weight cols per load group (768)
                for quarter in range(4):
                    wq = qwpool.tile([P, db, QW], bf16, tag="wq")
                    nc.sync.dma_start(
                        wq[:], wqkv[l, :, quarter * QW:
                                     (quarter + 1) * QW].rearrange(
                            "(o p) c -> p o c", p=P))
                    for blk in range(nqb // 4):
                        b = quarter * (nqb // 4) + blk
                        comp = b // NC        # 0=q, 1=k, 2=v (comp-major)
                        pq = ps.tile([P, t], f32, tag="ps")
                        for dbi in range(db):
                            nc.tensor.matmul(
                                pq[:], wq[:, dbi, blk * P:(blk + 1) * P],
                                xn[:, dbi, :],
                                start=(dbi == 0), stop=(dbi == db - 1))
                        if comp == 2:
                            nc.vector.tensor_copy(qkv_sb[:, b, :], pq[:])
                        else:
                            raw = epool.tile([P, t], bf16, tag="rope_raw")
                            nc.vector.tensor_copy(raw[:], pq[:])
                            prot = ps.tile([P, t], f32, tag="ps")
                            nc.tensor.matmul(prot[:], rot_t[:], raw[:],
                                             start=True, stop=True)
                            t1 = epool.tile([P, t], bf16, tag="rope_t1")
                            nc.vector.tensor_tensor(
                                t1[:], prot[:], sin_t[:],
                                mybir.AluOpType.mult)
                            t2 = epool.tile([P, t], bf16, tag="rope_t2")
                            nc.vector.tensor_tensor(
                                t2[:], raw[:], cos_t[:],
                                mybir.AluOpType.mult)
                            nc.vector.tensor_tensor(
                                qkv_sb[:, b, :], t1[:], t2[:],
                                mybir.AluOpType.add)
                for j in range(3):
                    nc.sync.dma_start(
                        a2a_in[:, j * P:(j + 1) * P, :]
                        .rearrange("c p t -> p c t"),
                        qkv_sb[:, j * NC:(j + 1) * NC, :])
                nc.gpsimd.collective_compute(
                    "AllToAll", mybir.AluOpType.bypass,
                    replica_groups=[list(range(NC))],
                    ins=[a2a_in.ap().opt()], outs=[a2a_out.ap().opt()])

                # ----- unpack: qT/kT [P, n]; V -> per-head [tok, dh|ones] -----
                qT = apool.tile([P, n], bf16, tag="qT")
                kT = apool.tile([P, n], bf16, tag="kT")
                vT = apool.tile([P, n], bf16, tag="vT")
                for dst, j in ((qT, 0), (kT, 1), (vT, 2)):
                    nc.sync.dma_start(
                        dst[:].rearrange("d (c t) -> d c t", c=NC),
                        a2a_out[:, j * P:(j + 1) * P, :]
                        .rearrange("c d t -> d c t"))
                for kbi in range(cfg.n_kb):
                    ptv = ps.tile([P, P], f32, tag="ps")
                    nc.tensor.matmul(
                        ptv[:], vT[:, kbi * P:(kbi + 1) * P], ident_bf[:],
                        start=True, stop=True)
                    for h in range(cfg.heads_per_core):
                        nc.vector.tensor_copy(
                            vh[h][:, kbi, :DH], ptv[:, h * DH:(h + 1) * DH])

                # ----- attention (2 heads), output attT [P, n] bf16 -----
                attT = apool.tile([P, n], bf16, tag="attT")
                for h in range(cfg.heads_per_core):
                    hsl = slice(h * DH, (h + 1) * DH)
                    for qc_i in range(cfg.n_qc):
                        qsl = slice(qc_i * cfg.qc_w, (qc_i + 1) * cfg.qc_w)
                        n_kb_q = (qc_i + 1) * cfg.kb_per_qc
                        pav = pacc.tile([DH + 1, cfg.qc_w], f32, tag="acc_a")
                        # kb blocks processed in pairs: QK lands in a 2-bank
                        # psum tile so exp runs as one wide ACT op per pair
                        for pi in range(n_kb_q // 2):
                            kb0 = 2 * pi
                            wide = pacc.tile(
                                [P, 2, cfg.qc_w], f32,
                                tag=f"acc_{'bc'[pi % 2]}",
                                name=f"qkp_{l}_{h}_{qc_i}_{pi}")
                            for s in range(2):
                                kbi = kb0 + s
                                nc.tensor.matmul(
                                    wide[:, s, :],
                                    kT[hsl, kbi * P:(kbi + 1) * P],
                                    qT[hsl, qsl], start=True, stop=True)
                            pe = epool.tile([P, 2, cfg.qc_w], bf16,
                                            tag="att_exp")
                            nc.scalar.activation(pe[:], wide[:], AF.Exp)
                            j = kb0 - qc_i * cfg.kb_per_qc
                            if j >= 0:
                                nc.vector.tensor_tensor(
                                    pe[:], pe[:], mask_t[:, j:j + 2, :],
                                    mybir.AluOpType.mult)
                            for s in range(2):
                                kbi = kb0 + s
                                nc.tensor.matmul(
                                    pav[:], vh[h][:, kbi, :], pe[:, s, :],
                                    start=(kbi == 0),
                                    stop=(kbi == n_kb_q - 1))
                        inv = spool.tile([1, cfg.qc_w], f32, tag="att_inv")
                        nc.vector.reciprocal(inv[:], pav[DH:DH + 1, :])
                        pb = ps.tile([DH, cfg.qc_w], f32, tag="ps")
                        nc.tensor.matmul(pb[:], ones_row[:, :DH], inv[:],
                                         start=True, stop=True)
                        invb = spool.tile([DH, cfg.qc_w], f32, tag="att_invb")
                        nc.vector.tensor_copy(invb[:], pb[:])
                        nc.vector.tensor_tensor(
                            attT[hsl, qsl], pav[:DH, :], invb[:],
                            mybir.AluOpType.mult)

                # ----- AllToAll back to token-local -----
                nc.sync.dma_start(
                    a2o_in[:, :, :].rearrange("c d t -> d c t"),
                    attT[:].rearrange("d (c t) -> d c t", c=NC))
                nc.gpsimd.collective_compute(
                    "AllToAll", mybir.AluOpType.bypass,
                    replica_groups=[list(range(NC))],
                    ins=[a2o_in.ap().opt()], outs=[a2o_out.ap().opt()])
                attC = wpool.tile([P, NC, t], bf16, tag="attC")
                nc.sync.dma_start(
                    attC[:], a2o_out[:, :, :].rearrange("c d t -> d c t"))

                # ----- out-proj + residual -----
                # contraction split in two halves of 4 hb blocks so the
                # weight tile double-buffers at half size; 8 outputs live in
                # 4 pair-packed psum accumulators across both halves
                pgo = [pacc.tile([P, 2, t], f32, tag=f"acc_{'abcd'[pi]}",
                                 name=f"oacc_{l}_{pi}") for pi in range(4)]
                for half in range(2):
                    wo = wtpool.tile([P, NC // 2, D], bf16, tag="wo")
                    nc.sync.dma_start(
                        wo[:], wout[l, half * (D // 2):(half + 1) * (D // 2),
                                    :].rearrange("(hb p) q -> p hb q", p=P))
                    for dci in range(db):
                        for hb in range(NC // 2):
                            nc.tensor.matmul(
                                pgo[dci // 2][:, dci % 2, :],
                                wo[:, hb, dci * P:(dci + 1) * P],
                                attC[:, half * 4 + hb, :],
                                start=(half == 0 and hb == 0),
                                stop=(half == 1 and hb == NC // 2 - 1))
                for dci in range(db):
                    nc.vector.tensor_tensor(hT[:, dci, :], hT[:, dci, :],
                                            pgo[dci // 2][:, dci % 2, :],
                                            mybir.AluOpType.add)

                # ----- norm2 + FFN (token-local, no collective) -----
                xn2 = wpool.tile([P, db, t], bf16, tag="xn")
                rms_norm_cast(hT, xn2)
                b1 = spool.tile([P, fb], f32, tag="b1")
                nc.sync.dma_start(b1[:], bff1[l, :, :])
                b2 = spool.tile([P, db], f32, tag="b2")
                nc.sync.dma_start(b2[:], bff2[l, :, :])
                # fused ff1 -> gelu -> ff2: 4 pair-psum accumulators hold the
                # 8 D-chunk outputs; act chunk is transient. Weights stream in
                # 512-ff-col groups (contiguous >=1KB DMA elements).
                pgs = []
                for pi in range(4):
                    pg_i = pacc.tile([P, 2, t], f32, tag=f"acc_{'abcd'[pi]}",
                                     name=f"ffacc_{l}_{pi}")
                    pgs.append(pg_i)
                GW = 512            # ff cols per weight-load group
                ng = ff // GW
                fpg = GW // P       # fci per group (4)
                for gi in range(ng):
                    w1g = wtpool.tile([P, db, GW], bf16, tag="w1g")
                    nc.sync.dma_start(
                        w1g[:], wff1[l, :, gi * GW:(gi + 1) * GW].rearrange(
                            "(o p) c -> p o c", p=P))
                    w2g = wtpool.tile([P, fpg, db, P], bf16, tag="w2g")
                    for fi in range(fpg):
                        nc.sync.dma_start(
                            w2g[:, fi, :, :],
                            wff2[l, (gi * fpg + fi) * P:
                                 (gi * fpg + fi + 1) * P, :].rearrange(
                                "p (dc q) -> p dc q", q=P))
                    for fi in range(fpg):
                        fci = gi * fpg + fi
                        pf = ps.tile([P, t], f32, tag="ps")
                        for dbi in range(db):
                            nc.tensor.matmul(
                                pf[:], w1g[:, dbi, fi * P:(fi + 1) * P],
                                xn2[:, dbi, :],
                                start=(dbi == 0), stop=(dbi == db - 1))
                        act_c = epool.tile([P, t], bf16, tag="act_c")
                        nc.scalar.activation(act_c[:], pf[:], AF.Gelu,
                                             bias=b1[:, fci:fci + 1])
                        for dci in range(db):
                            nc.tensor.matmul(
                                pgs[dci // 2][:, dci % 2, :],
                                w2g[:, fi, dci, :], act_c[:],
                                start=(fci == 0 and dci % 2 == 0),
                                stop=(fci == fb - 1 and dci % 2 == 1))
                for dci in range(db):
                    tmp = spool.tile([P, t], f32, tag="ff2_tmp")
                    nc.scalar.activation(tmp[:], pgs[dci // 2][:, dci % 2, :],
                                         AF.Identity, bias=b2[:, dci:dci + 1])
                    nc.vector.tensor_tensor(hT[:, dci, :], hT[:, dci, :],
                                            tmp[:], mybir.AluOpType.add)

            # ================= final norm + logits =================
            xnf = wpool.tile([P, db, t], bf16, tag="xn")
            rms_norm_cast(hT, xnf)
            vchunks = []
            off = 0
            while off < v:
                w = min(512, v - off)
                vchunks.append((off, w))
                off += w
            for (off, w) in vchunks:
                wl = wtpool.tile([P, db, 512], bf16, tag="wl")
                nc.sync.dma_start(
                    wl[:, :, :w],
                    wlog[:, off:off + w].rearrange("(o p) c -> p o c", p=P))
                for tbi in range(tb):
                    pl = pacc.tile([P, 512], f32, tag=f"acc_{'ab'[tbi % 2]}",
                                   name=f"pl_{off}_{tbi}")
                    for dbi in range(db):
                        nc.tensor.matmul(
                            pl[:, :w], xnf[:, dbi, tbi * P:(tbi + 1) * P],
                            wl[:, dbi, :w],
                            start=(dbi == 0), stop=(dbi == db - 1))
                    ot = spool.tile([P, 512], bf16, tag="log_out")
                    nc.vector.tensor_copy(ot[:, :w], pl[:, :w])
                    nc.sync.dma_start(
                        logits_out[tbi * P:(tbi + 1) * P, off:off + w],
                        ot[:, :w])
    nc.finalize()
    return nc


# ======================= host side =======================

def prep_inputs(inputs, cfg=FULL):
    """Full model inputs -> list of 8 per-core input dicts (numpy)."""
    n, t, depth, v, ff = cfg.n, cfg.t, cfg.depth, cfg.v, cfg.ff
    x = np.asarray(inputs["x"]).reshape(-1)[:n].astype(np.int32)
    emb = np.asarray(inputs["token_emb"], dtype=np.float32)[:v]
    attn_g = np.asarray(inputs["attn_gamma"], dtype=np.float32)
    w_qkv = np.asarray(inputs["w_qkv"], dtype=np.float32)
    w_out = np.asarray(inputs["w_attn_out"], dtype=np.float32)
    ff_g = np.asarray(inputs["ff_gamma"], dtype=np.float32)
    w_ff1 = np.asarray(inputs["w_ff1"], dtype=np.float32)[:, :, :ff]
    b_ff1 = np.asarray(inputs["b_ff1"], dtype=np.float32)[:, :ff]
    w_ff2 = np.asarray(inputs["w_ff2"], dtype=np.float32)[:, :ff, :]
    b_ff2 = np.asarray(inputs["b_ff2"], dtype=np.float32)
    fin_g = np.asarray(inputs["final_gamma"], dtype=np.float32)
    w_log = np.asarray(inputs["w_logits"], dtype=np.float32)[:, :v]

    sD = float(np.sqrt(D))
    # rope tables
    inv_freq = 1.0 / (10000.0 ** (np.arange(0, DH, 2, dtype=np.float32) / DH))
    freqs = np.arange(n, dtype=np.float32)[:, None] * inv_freq[None, :]
    freqs = np.repeat(freqs, 2, axis=-1)          # [n, DH]
    cos = np.cos(freqs).T                          # [DH, n]
    sin = np.sin(freqs).T
    cos2 = np.tile(cos, (2, 1)).astype(BF16)       # [128, n]
    sin2 = np.tile(sin, (2, 1)).astype(BF16)
    # rot matrix PT st rot(x) = PT.T @ x, block-diag per head (DH x DH)
    Pm = np.zeros((DH, DH), np.float32)
    for i in range(0, DH, 2):
        Pm[i, i + 1] = -1.0
        Pm[i + 1, i] = 1.0
    PT1 = Pm.T
    PT = np.zeros((P, P), np.float32)
    for h in range(2):
        PT[h * DH:(h + 1) * DH, h * DH:(h + 1) * DH] = PT1
    PT = PT.astype(BF16)
    # causal masks for diagonal kb of each q-chunk
    tri = np.zeros((cfg.kb_per_qc, P, cfg.qc_w), np.float32)
    qpos = np.arange(cfg.qc_w)
    for j in range(cfg.kb_per_qc):
        kpos = j * P + np.arange(P)
        tri[j] = (kpos[:, None] <= qpos[None, :]).astype(np.float32)
    tri = tri.astype(BF16)

    scale_q = DH ** -0.5
    # qkv weights, shared across cores: cols regrouped dest-major
    wqkv_l = []
    for l in range(cfg.depth):
        g = attn_g[l] * sD
        wqkv3 = w_qkv[l].reshape(D, 3, H * DH)
        wqf = (g[:, None] * wqkv3[:, 0, :]) * scale_q    # [D, H*DH]
        wkf = g[:, None] * wqkv3[:, 1, :]
        wvf = g[:, None] * wqkv3[:, 2, :]
        # component-major: cols = [q all heads | k all heads | v all heads]
        wqkv_l.append(np.concatenate([wqf, wkf, wvf], axis=1))  # [D, 3*H*DH]
    wqkv_all = np.stack(wqkv_l, 0).astype(BF16)

    wo_l, w1_l, w2_l, b1_l, b2_l = [], [], [], [], []
    for l in range(cfg.depth):
        wo_l.append(w_out[l])
        gf = ff_g[l] * sD
        w1_l.append(gf[:, None] * w_ff1[l])
        w2_l.append(w_ff2[l])
        b1_l.append(b_ff1[l].reshape(cfg.fb, P).T)
        b2_l.append(b_ff2[l].reshape(cfg.db, P).T)
    wlogg = (fin_g * sD)[:, None] * w_log

    shared = {
        "emb_tab": emb,
        "wqkv": wqkv_all,
        "wout": np.stack(wo_l, 0).astype(BF16),
        "wff1": np.stack(w1_l, 0).astype(BF16),
        "bff1": np.stack(b1_l, 0).astype(np.float32),
        "wff2": np.stack(w2_l, 0).astype(BF16),
        "bff2": np.stack(b2_l, 0).astype(np.float32),
        "wlog": wlogg.astype(BF16),
        "rotPT": PT, "trimask": tri,
    }
    in_maps = []
    for c in range(NC):
        d = dict(shared)
        d["emb_idx"] = x[c * t:(c + 1) * t].reshape(t, 1)
        d["cosq"] = np.ascontiguousarray(cos2[:, c * t:(c + 1) * t])
        d["sinq"] = np.ascontiguousarray(sin2[:, c * t:(c + 1) * t])
        in_maps.append(d)
    return in_maps


_CACHED = {}


def kernel(**inputs):
    import jax
    from jax.sharding import Mesh, PartitionSpec
    from jax.experimental.shard_map import shard_map
    from concourse.bass2jax import (_bass_exec_p, install_neuronx_cc_hook,
                                    partition_id_tensor)
    cfg = FULL
    in_maps = prep_inputs(inputs, cfg)
    if "nc" not in _CACHED:
        _CACHED["nc"] = build_kernel(cfg)
    nc = _CACHED["nc"]
    install_neuronx_cc_hook()
    partition_name = nc.partition_id_tensor.name if nc.partition_id_tensor else None
    in_names, out_names, out_avals = [], [], []
    for alloc in nc.m.functions[0].allocations:
        if not isinstance(alloc, mybir.MemoryLocationSet):
            continue
        name = alloc.memorylocations[0].name
        if alloc.kind == "ExternalInput":
            if name != partition_name:
                in_names.append(name)
        elif alloc.kind == "ExternalOutput":
            out_names.append(name)
            out_avals.append(jax.core.ShapedArray(
                tuple(alloc.tensor_shape), mybir.dt.np(alloc.dtype)))
    all_in = list(in_names) + list(out_names)
    if partition_name is not None:
        all_in.append(partition_name)
    n_params = len(in_names)

    def _body(*args):
        operands = list(args)
        if partition_name is not None:
            operands.append(partition_id_tensor())
        return tuple(_bass_exec_p.bind(
            *operands, out_avals=tuple(out_avals), in_names=tuple(all_in),
            out_names=tuple(out_names), lowering_input_output_aliases=(),
            sim_require_finite=True, sim_require_nnan=True, nc=nc))

    devices = jax.devices()[:NC]
    mesh = Mesh(np.asarray(devices), ("core",))
    n_outs = len(out_names)
    sharded = jax.jit(
        shard_map(_body, mesh=mesh,
                  in_specs=(PartitionSpec("core"),) * (n_params + n_outs),
                  out_specs=(PartitionSpec("core"),) * n_outs,
                  check_rep=False),
        donate_argnums=tuple(range(n_params, n_params + n_outs)),
        keep_unused=True)
    concat_in = [np.concatenate([np.asarray(in_maps[c][nm]) for c in range(NC)], 0)
                 for nm in in_names]
    zeros = [np.zeros((NC * a.shape[0], *a.shape[1:]), a.dtype) for a in out_avals]
    out = sharded(*concat_in, *zeros)
    logits = np.asarray(out[out_names.index("logits_out")])
    return logits.reshape(B, cfg.n, cfg.v).astype(np.float32)
